# revision 1
# baseline (speedup 1.0000x reference)
"""Trainium2 Bass kernel for NodeNetworkG GNN message passing.

Algorithm (8 NeuronCores, SPMD, no collectives):
  - Nodes are sorted by total degree and dealt round-robin to 8 cores, so each
    core owns ~1/8 of the nodes AND ~1/8 of the edges for both aggregation
    directions (mi: group by col, mo: group by row).
  - Per core, owned destinations are packed into 128-node blocks; each block
    gets S slot-columns (S = max degree in block, rounded up to even). Edges
    are laid out host-side as [128, ncols] int32 gather-index / f32 weight
    arrays; padded slots point at a zeros row of the x table with weight 0.
  - Device: indirect-DMA gathers x rows per slot, DVE multiplies by edge
    weight, then a pairwise fold tree sums the S slots per destination,
    yielding mi/mo [128, nb*48] resident in SBUF.
  - Node-wise MLP: PE transposes [mi|mo|x] tiles to feature-major, two matmuls
    with tanh activations (bias via ACT per-partition bias), output written
    transposed; host restores order.
"""

import numpy as np

P = 128
NCORES = 8
DIN = 48
DHID = 128

_PROG_CACHE: dict = {}


def _round_up(a, m):
    return (a + m - 1) // m * m


def _slot_layout(S_blk):
    """Given per-block slot counts (true block order), build the direction
    layout: blocks sorted by S desc; returns (ord_blocks, pos, col_off, total_cols,
    groups) where groups = list of (S, nbs, col_base, out_base_blkpos) runs of
    equal S split later into subchunks."""
    nb = len(S_blk)
    ordb = sorted(range(nb), key=lambda b: -S_blk[b])
    pos = [0] * nb
    for j, b in enumerate(ordb):
        pos[b] = j
    col_off = [0] * nb  # column offset (in slot units) of j-th block in ordb
    total = 0
    for j, b in enumerate(ordb):
        col_off[j] = total
        total += S_blk[b]
    # group runs of equal S (in ordb order)
    groups = []
    j = 0
    while j < nb:
        S = S_blk[ordb[j]]
        j0 = j
        while j < nb and S_blk[ordb[j]] == S:
            j += 1
        groups.append((S, j0, j - j0))  # S, first block pos, n blocks
    return ordb, pos, col_off, total, groups


def _host_prep(x, edge_index, edge_attr):
    N = x.shape[0]
    row = np.asarray(edge_index[0]).astype(np.int32)
    col = np.asarray(edge_index[1]).astype(np.int32)
    w = np.asarray(edge_attr, dtype=np.float32).reshape(-1)
    E = row.shape[0]

    npad = _round_up(N, P * NCORES)
    nloc = npad // NCORES
    nb = nloc // P

    deg_in = np.bincount(col, minlength=npad)
    deg_out = np.bincount(row, minlength=npad)
    order = np.argsort(-(deg_in + deg_out), kind="stable")
    rank = np.empty(npad, np.int64)
    rank[order] = np.arange(npad)
    core = (rank % NCORES).astype(np.int32)

    def direction_maps(deg):
        """Per-core block packing sorted by this direction's degree.
        Returns blk[node], part[node], S_b (unified over cores)."""
        blk = np.empty(npad, np.int32)
        part = np.empty(npad, np.int32)
        for k in range(NCORES):
            nodes_k = np.where(core == k)[0]
            lk = nodes_k[np.argsort(-deg[nodes_k], kind="stable")]
            pos = np.arange(nloc)
            blk[lk] = (pos // P).astype(np.int32)
            part[lk] = (pos % P).astype(np.int32)
        m = np.zeros(nb, np.int64)
        np.maximum.at(m, blk, deg)
        S = m
        return blk, part, S

    blk_i, part_i, S_in = direction_maps(deg_in)
    blk_o, part_o, S_out = direction_maps(deg_out)

    lay_in = _slot_layout(list(S_in))
    lay_out = _slot_layout(list(S_out))
    CM = lay_in[3]
    CO = lay_out[3]

    ZROW = N  # zeros row index in x_tab

    def build_dir(dest, src, lay, blk, part):
        """idx/w arrays [NCORES, P, C] for one direction."""
        _ordb, pos, col_off, C, _groups = lay
        idx = np.full((NCORES, P, C), ZROW, np.int32)
        wv = np.zeros((NCORES, P, C), np.float32)
        # slot rank within destination
        sortp = np.argsort(dest, kind="stable")
        dsort = dest[sortp]
        deg = np.bincount(dest, minlength=npad)
        first = np.cumsum(deg) - deg
        srank = np.arange(E) - first[dsort]
        e = sortp  # original edge ids in dest-sorted order
        d = dsort
        c_e = core[d]
        b_e = blk[d]
        p_e = part[d]
        colpos = np.asarray(col_off, np.int64)
        posarr = np.asarray(pos, np.int64)
        cidx = colpos[posarr[b_e]] + srank
        idx[c_e, p_e, cidx] = src[e]
        wv[c_e, p_e, cidx] = w[e]
        return idx, wv

    idx_mi, w_mi = build_dir(col, row, lay_in, blk_i, part_i)
    idx_mo, w_mo = build_dir(row, col, lay_out, blk_o, part_o)

    # x table with zeros row
    x_tab = np.zeros((N + 1, DIN), np.float32)
    x_tab[:N] = np.asarray(x, np.float32)

    # per-core x shard, rows in mi-local order (b*128+p); dummies zero
    x_pad = np.zeros((npad + 1, DIN), np.float32)
    x_pad[:N] = np.asarray(x, np.float32)
    x_own = np.zeros((NCORES, nloc, DIN), np.float32)

    # realign indices: for mi-local slot (c=blk_i, p=part_i) of node n, the row
    # of n in the mo scratch layout (row = pos_out[blk_o]*128 + part_o)
    pos_out = np.asarray(lay_out[1], np.int64)
    realign = np.zeros((NCORES, P, nb), np.int32)
    realign[core, part_i, blk_i] = (pos_out[blk_o] * P + part_o).astype(np.int32)

    # output row mapping: out_t column (b*128+p) of core k = node at mi-local
    nodes_of_core = []
    for k in range(NCORES):
        lk = np.full(nloc, npad, np.int64)
        sel = core == k
        lk[blk_i[sel].astype(np.int64) * P + part_i[sel]] = np.where(sel)[0]
        nodes_of_core.append(lk)
        x_own[k] = x_pad[lk]

    meta = dict(
        N=N,
        E=E,
        npad=npad,
        nb=nb,
        CM=CM,
        CO=CO,
        lay_in=lay_in,
        lay_out=lay_out,
        x_tab=x_tab,
        idx_mi=idx_mi,
        w_mi=w_mi,
        idx_mo=idx_mo,
        w_mo=w_mo,
        x_own=x_own,
        realign=realign,
        nodes_of_core=nodes_of_core,
    )
    return meta


def _round_up_arr(a, m):
    return (a + m - 1) // m * m


def _subchunks(lay, CC):
    """Split the direction layout into subchunks:
    (S, nbs, col_base_slots, out_blkpos) with S*nbs <= CC."""
    _ordb, _pos, col_off, _total, groups = lay
    out = []
    for S, j0, njb in groups:
        if S == 0:
            continue
        nbs_max = max(1, CC // S)
        j = 0
        while j < njb:
            nbs = min(nbs_max, njb - j)
            out.append((S, nbs, col_off[j0 + j], j0 + j))
            j += nbs
    return out


def _build_program(meta, CC=64):
    import concourse.bacc as bacc
    import concourse.bass as bass
    import concourse.mybir as mybir
    import concourse.tile as tile
    from concourse.masks import make_identity

    N = meta["N"]
    nb = meta["nb"]
    CM = meta["CM"]
    CO = meta["CO"]
    f32 = mybir.dt.float32
    i32 = mybir.dt.int32

    nc = bacc.Bacc(
        "TRN2",
        target_bir_lowering=False,
        debug=False,
        num_devices=NCORES,
        dynamic_dma_scratch_size=65536,
    )

    x_tab = nc.dram_tensor("x_tab", [N + 1, DIN], f32, kind="ExternalInput")
    idx_mi_d = nc.dram_tensor("idx_mi", [P, CM], i32, kind="ExternalInput")
    w_mi_d = nc.dram_tensor("w_mi", [P, CM], f32, kind="ExternalInput")
    idx_mo_d = nc.dram_tensor("idx_mo", [P, CO], i32, kind="ExternalInput")
    w_mo_d = nc.dram_tensor("w_mo", [P, CO], f32, kind="ExternalInput")
    x_own_d = nc.dram_tensor("x_own", [nb * P, DIN], f32, kind="ExternalInput")
    realign_d = nc.dram_tensor("realign", [P, nb], i32, kind="ExternalInput")
    mo_scratch = nc.dram_tensor("mo_scratch", [nb * P, DIN], f32, kind="Internal")
    w1ta_d = nc.dram_tensor("w1ta", [DIN, DHID], f32, kind="ExternalInput")
    w1tb_d = nc.dram_tensor("w1tb", [DIN, DHID], f32, kind="ExternalInput")
    w1tc_d = nc.dram_tensor("w1tc", [DIN, DHID], f32, kind="ExternalInput")
    w2t_d = nc.dram_tensor("w2t", [DHID, DHID], f32, kind="ExternalInput")
    b1_d = nc.dram_tensor("b1", [DHID, 1], f32, kind="ExternalInput")
    b2_d = nc.dram_tensor("b2", [DHID, 1], f32, kind="ExternalInput")
    out_t = nc.dram_tensor("out_t", [P, nb * P], f32, kind="ExternalOutput")

    with tile.TileContext(nc) as tc:
        with (
            tc.tile_pool(name="const", bufs=1) as const,
            tc.tile_pool(name="gpool", bufs=3) as gpool,
            tc.tile_pool(name="mlp", bufs=3) as mlp,
            tc.tile_pool(name="ost", bufs=2) as ostp,
            tc.tile_pool(name="psA", bufs=3, space="PSUM") as psA,
            tc.tile_pool(name="psH", bufs=2, space="PSUM") as psH,
        ):
            # ---- load constants / index arrays ----
            idx_mi_sb = const.tile([P, CM], i32)
            nc.sync.dma_start(idx_mi_sb[:], idx_mi_d[:])
            w_mi_sb = const.tile([P, CM], f32)
            nc.sync.dma_start(w_mi_sb[:], w_mi_d[:])
            idx_mo_sb = const.tile([P, CO], i32)
            nc.sync.dma_start(idx_mo_sb[:], idx_mo_d[:])
            w_mo_sb = const.tile([P, CO], f32)
            nc.sync.dma_start(w_mo_sb[:], w_mo_d[:])
            realign_sb = const.tile([P, nb], i32)
            nc.sync.dma_start(realign_sb[:], realign_d[:])
            w1ta_sb = const.tile([DIN, DHID], f32)
            nc.sync.dma_start(w1ta_sb[:], w1ta_d[:])
            w1tb_sb = const.tile([DIN, DHID], f32)
            nc.sync.dma_start(w1tb_sb[:], w1tb_d[:])
            w1tc_sb = const.tile([DIN, DHID], f32)
            nc.sync.dma_start(w1tc_sb[:], w1tc_d[:])
            w2t_sb = const.tile([DHID, DHID], f32)
            nc.sync.dma_start(w2t_sb[:], w2t_d[:])
            b1_sb = const.tile([DHID, 1], f32)
            nc.sync.dma_start(b1_sb[:], b1_d[:])
            b2_sb = const.tile([DHID, 1], f32)
            nc.sync.dma_start(b2_sb[:], b2_d[:])
            ident = const.tile([P, P], f32)
            make_identity(nc, ident[:])

            mi_sb = const.tile([P, nb * DIN], f32)
            mo_sb = const.tile([P, nb * DIN], f32)
            mo2_sb = const.tile([P, nb * DIN], f32)
            xo_sb = const.tile([P, nb * DIN], f32)

            # own-x shard arrives pre-permuted: one strided DMA
            nc.sync.dma_start(
                xo_sb[:].rearrange("p (j f) -> p j f", f=DIN),
                x_own_d[:].rearrange("(j p) f -> p j f", p=P),
            )

            # ---- aggregation ----
            for lay, idx_sb, w_sb, acc_sb in (
                (meta["lay_out"], idx_mo_sb, w_mo_sb, mo_sb),
                (meta["lay_in"], idx_mi_sb, w_mi_sb, mi_sb),
            ):
                # zero-degree tail blocks: memset their accumulator columns
                zblocks = [j for j, b in enumerate(lay[0]) if _S_of(lay, j) == 0]
                if zblocks:
                    z0 = min(zblocks)
                    nzb = len(zblocks)
                    nc.vector.memset(acc_sb[:, z0 * DIN : (z0 + nzb) * DIN], 0.0)
                for S, nbs, c0, outpos in _subchunks(lay, CC):
                    cols = S * nbs
                    G = gpool.tile([P, CC * DIN], f32, tag="G")
                    g3 = G[:, : cols * DIN].rearrange("p (c f) -> p c f", f=DIN)
                    for c in range(cols):
                        nc.gpsimd.indirect_dma_start(
                            out=G[:, c * DIN : (c + 1) * DIN],
                            out_offset=None,
                            in_=x_tab[:],
                            in_offset=bass.IndirectOffsetOnAxis(
                                ap=idx_sb[:, c0 + c : c0 + c + 1], axis=0
                            ),
                        )
                    # multiply by per-slot weight (broadcast over feature dim)
                    wv = w_sb[:, c0 : c0 + cols]
                    wb = bass.AP(
                        wv.tensor,
                        wv.offset,
                        [list(wv.ap[0]), list(wv.ap[1]), [0, DIN]],
                    )
                    nc.vector.tensor_tensor(
                        out=g3, in0=g3, in1=wb, op=mybir.AluOpType.mult
                    )
                    # fold S slots -> slot 0 (pairwise tree), per block
                    g4 = G[:, : cols * DIN].rearrange(
                        "p (b s f) -> p b s f", s=S, f=DIN
                    )
                    s = S
                    while s > 1:
                        half = s // 2
                        hi0 = s - half
                        nc.vector.tensor_tensor(
                            out=g4[:, :, 0:half, :],
                            in0=g4[:, :, 0:half, :],
                            in1=g4[:, :, hi0:s, :],
                            op=mybir.AluOpType.add,
                        )
                        s = hi0
                    # copy folded result to accumulator columns
                    nc.vector.tensor_copy(
                        out=acc_sb[:, outpos * DIN : (outpos + nbs) * DIN].rearrange(
                            "p (b f) -> p b f", f=DIN
                        ),
                        in_=g4[:, :, 0, :],
                    )

                if acc_sb is mo_sb:
                    # realign mo to mi-local order via DRAM scratch, before the
                    # mi phase so MLP tiles overlap the mi gather stream
                    nc.sync.dma_start(
                        mo_scratch[:].rearrange("(j p) f -> p j f", p=P),
                        mo_sb[:].rearrange("p (j f) -> p j f", f=DIN),
                    )
                    for c in range(nb):
                        nc.gpsimd.indirect_dma_start(
                            out=mo2_sb[:, c * DIN : (c + 1) * DIN],
                            out_offset=None,
                            in_=mo_scratch[:],
                            in_offset=bass.IndirectOffsetOnAxis(
                                ap=realign_sb[:, c : c + 1], axis=0
                            ),
                        )

            # ---- MLP over 128-node tiles (mi-local order) ----
            pos_in = meta["lay_in"][1]
            OG = 4  # output tiles per DMA group
            for b0 in range(0, nb, OG):
                og = min(OG, nb - b0)
                os_ = ostp.tile([P, OG * P], f32, tag="os")
                for j in range(og):
                    b = b0 + j
                    mi_c = mi_sb[:, pos_in[b] * DIN : (pos_in[b] + 1) * DIN]
                    mo_c = mo2_sb[:, b * DIN : (b + 1) * DIN]
                    xo_c = xo_sb[:, b * DIN : (b + 1) * DIN]
                    hp = psH.tile([P, P], f32, tag="hp")
                    for q, (src_c, w1q) in enumerate(
                        ((mi_c, w1ta_sb), (mo_c, w1tb_sb), (xo_c, w1tc_sb))
                    ):
                        pA = psA.tile([DIN, P], f32, tag="pA")
                        nc.tensor.transpose(pA[:], src_c, ident[:])
                        mt = mlp.tile([DIN, P], f32, tag="mt")
                        nc.vector.tensor_copy(out=mt[:], in_=pA[:])
                        nc.tensor.matmul(
                            hp[:], w1q[:], mt[:], start=(q == 0), stop=(q == 2)
                        )
                    hs = mlp.tile([P, P], f32, tag="hs")
                    nc.scalar.activation(
                        hs[:],
                        hp[:],
                        mybir.ActivationFunctionType.Tanh,
                        bias=b1_sb[:],
                        scale=1.0,
                    )
                    op_ = psH.tile([P, P], f32, tag="op")
                    nc.tensor.matmul(op_[:], w2t_sb[:], hs[:], start=True, stop=True)
                    nc.scalar.activation(
                        os_[:, j * P : (j + 1) * P],
                        op_[:],
                        mybir.ActivationFunctionType.Tanh,
                        bias=b2_sb[:],
                        scale=1.0,
                    )
                nc.sync.dma_start(
                    out_t[:, b0 * P : (b0 + og) * P], os_[:, : og * P]
                )

    nc.compile()
    return nc


def _S_of(lay, j):
    """Slot count of j-th block (in ordb order)."""
    ordb = lay[0]
    col_off = lay[2]
    total = lay[3]
    nxt = col_off[j + 1] if j + 1 < len(col_off) else total
    return nxt - col_off[j]


def kernel(x, edge_index, edge_attr, W1, b1, W2, b2):
    x = np.asarray(x, np.float32)
    meta = _host_prep(x, edge_index, edge_attr)
    key = (meta["N"], meta["E"], meta["nb"], meta["CM"], meta["CO"],
           tuple(meta["lay_in"][2]), tuple(meta["lay_out"][2]))
    if key not in _PROG_CACHE:
        _PROG_CACHE[key] = _build_program(meta)
    nc = _PROG_CACHE[key]

    W1 = np.asarray(W1, np.float32)
    W2 = np.asarray(W2, np.float32)
    b1 = np.asarray(b1, np.float32).reshape(DHID, 1)
    b2 = np.asarray(b2, np.float32).reshape(DHID, 1)
    w1t = np.ascontiguousarray(W1.T)  # [144, 128]
    w1ta = np.ascontiguousarray(w1t[:DIN])
    w1tb = np.ascontiguousarray(w1t[DIN : 2 * DIN])
    w1tc = np.ascontiguousarray(w1t[2 * DIN :])
    w2t = np.ascontiguousarray(W2.T)

    in_maps = []
    for k in range(NCORES):
        in_maps.append(
            {
                "x_tab": meta["x_tab"],
                "idx_mi": meta["idx_mi"][k],
                "w_mi": meta["w_mi"][k],
                "idx_mo": meta["idx_mo"][k],
                "w_mo": meta["w_mo"][k],
                "x_own": meta["x_own"][k],
                "realign": meta["realign"][k],
                "w1ta": w1ta,
                "w1tb": w1tb,
                "w1tc": w1tc,
                "w2t": w2t,
                "b1": b1,
                "b2": b2,
            }
        )

    runner = _get_runner(nc)
    results = runner.run(in_maps)
    global _LAST
    _LAST = (nc, in_maps)

    N = meta["N"]
    out = np.empty((meta["npad"], DHID), np.float32)
    for k in range(NCORES):
        out[meta["nodes_of_core"][k]] = results[k]["out_t"].T
    return out[:N]


_LAST = None
_RUNNER_CACHE: dict = {}


class _PjrtRunner:
    """Builds the shard_map-jitted NEFF executor once; supports repeated
    dispatches with device-resident inputs for timing."""

    def __init__(self, nc):
        import jax
        import jax.numpy as jnp
        import concourse.mybir as mybir
        from concourse import bass2jax
        from jax.sharding import Mesh, NamedSharding, PartitionSpec
        from jax.experimental.shard_map import shard_map

        bass2jax.install_neuronx_cc_hook()
        self.jax = jax
        self.jnp = jnp
        in_names: list[str] = []
        out_names: list[str] = []
        out_avals = []
        out_shapes = []
        partition_name = (
            nc.partition_id_tensor.name if nc.partition_id_tensor else None
        )
        for alloc in nc.m.functions[0].allocations:
            if not isinstance(alloc, mybir.MemoryLocationSet):
                continue
            name = alloc.memorylocations[0].name
            if alloc.kind == "ExternalInput":
                if name != partition_name:
                    in_names.append(name)
            elif alloc.kind == "ExternalOutput":
                shape = tuple(alloc.tensor_shape)
                dtype = mybir.dt.np(alloc.dtype)
                out_names.append(name)
                out_avals.append(jax.core.ShapedArray(shape, dtype))
                out_shapes.append((shape, dtype))
        self.in_names = in_names
        self.out_names = out_names
        self.out_shapes = out_shapes
        n_params = len(in_names)
        n_outs = len(out_names)
        all_names = in_names + out_names
        if partition_name is not None:
            all_names = all_names + [partition_name]

        def _body(*args):
            operands = list(args)
            if partition_name is not None:
                operands.append(bass2jax.partition_id_tensor())
            outs = bass2jax._bass_exec_p.bind(
                *operands,
                out_avals=tuple(out_avals),
                in_names=tuple(all_names),
                out_names=tuple(out_names),
                lowering_input_output_aliases=(),
                sim_require_finite=True,
                sim_require_nnan=True,
                nc=nc,
            )
            return tuple(outs)

        devices = jax.devices()[:NCORES]
        self.mesh = Mesh(np.asarray(devices), ("core",))
        spec = PartitionSpec("core")
        self.sharding = NamedSharding(self.mesh, spec)
        self.sharded = jax.jit(
            shard_map(
                _body,
                mesh=self.mesh,
                in_specs=(spec,) * (n_params + n_outs),
                out_specs=(spec,) * n_outs,
                check_rep=False,
            ),
            donate_argnums=tuple(range(n_params, n_params + n_outs)),
            keep_unused=True,
        )

        def _mk_zeros():
            return tuple(
                jnp.zeros((NCORES * s[0], *s[1:]), d) for s, d in out_shapes
            )

        self.zeros_fn = jax.jit(
            _mk_zeros, out_shardings=(self.sharding,) * n_outs
        )

    def _stage_inputs(self, in_maps):
        concat = [
            np.concatenate([np.asarray(in_maps[c][n]) for c in range(NCORES)], axis=0)
            for n in self.in_names
        ]
        return [self.jax.device_put(a, self.sharding) for a in concat]

    def _dispatch(self, staged):
        zeros = self.zeros_fn()
        outs = self.sharded(*staged, *zeros)
        self.jax.block_until_ready(outs)
        return outs

    def run(self, in_maps):
        staged = self._stage_inputs(in_maps)
        outs = self._dispatch(staged)
        res = []
        for c in range(NCORES):
            m = {}
            for i, n in enumerate(self.out_names):
                s, d = self.out_shapes[i]
                m[n] = np.asarray(outs[i]).reshape(NCORES, *s)[c]
            res.append(m)
        return res

    def timed(self, in_maps, iters=10):
        """Wall-clock per dispatch (tunnel RTT included) minus a tiny-dispatch
        baseline; min-statistics over iters. Noisy through the axon tunnel —
        treat as an upper-bound cross-check of the cost model."""
        import time

        staged = self._stage_inputs(in_maps)
        self._dispatch(staged)  # warm
        walls = []
        for _ in range(iters):
            zeros = self.zeros_fn()
            self.jax.block_until_ready(zeros)
            t0 = time.perf_counter()
            outs = self.sharded(*staged, *zeros)
            self.jax.block_until_ready(outs)
            walls.append(time.perf_counter() - t0)
        tiny = self.jax.device_put(np.zeros((NCORES, 8), np.float32), self.sharding)
        base_fn = self.jax.jit(lambda a: a + 1.0)
        self.jax.block_until_ready(base_fn(tiny))
        bases = []
        for _ in range(iters):
            t0 = time.perf_counter()
            self.jax.block_until_ready(base_fn(tiny))
            bases.append(time.perf_counter() - t0)
        print(
            f"kernel walls min/med: {min(walls)*1e3:.2f}/{np.median(walls)*1e3:.2f} ms; "
            f"baseline min/med: {min(bases)*1e3:.2f}/{np.median(bases)*1e3:.2f} ms"
        )
        return max(float(np.median(walls) - np.median(bases)), 0.0) * 1e9


def _get_runner(nc):
    r = _RUNNER_CACHE.get(id(nc))
    if r is None:
        r = _PjrtRunner(nc)
        _RUNNER_CACHE[id(nc)] = r
    return r


def time_kernel(inputs=None, iters=8):
    """Median wall-clock of a kernel dispatch minus tiny-dispatch baseline, ns.
    Must be called after kernel()."""
    assert _LAST is not None, "call kernel() first"
    nc, in_maps = _LAST
    return _get_runner(nc).timed(in_maps, iters=iters)



# revision 18
# speedup vs baseline: 2.8203x; 2.8203x over previous
"""Trainium2 Bass kernel for NodeNetworkG GNN message passing.

Algorithm (8 NeuronCores, SPMD, no collectives):
  - Nodes are sorted by total degree and dealt round-robin to 8 cores; each
    core owns ~1/8 of nodes and edges for both aggregation directions.
  - Per direction, destinations are packed into 128-node blocks (sorted by
    that direction's degree). Each edge is assigned to one of 8 "classes"
    (4 node-id range chunks + 4 mod-4 decimated copies) via 2-choice load
    balancing, so every gather instruction's int16 indices address a
    25088-row fp16 table (dma_gather is limited to 1024 indices/instruction).
  - Bulk tier: per (block, class) band with T slots per destination (T chosen
    per band to trade slot padding vs overflow, quantized over block groups).
    Batched dma_gather (96B rows from 256B-stride tables), DVE weight-multiply
    and uniform-T fold in fp16, accumulated into [128, nb*48] accumulators.
  - Overflow tier (edges beyond T): class-grouped dma_gather at full packing,
    weight-multiply, then dma_scatter_add into a DRAM accumulator (rank-sliced
    windows keep destinations unique per scatter instruction).
  - mo is realigned to mi-local node order via the DRAM accumulator and one
    dma_gather. Node-wise MLP per 128-node block: PE transpose to
    feature-major, two fp16 matmuls with tanh (ACT bias), written
    feature-major; host restores order.
"""

import numpy as np

P = 128
NCORES = 8
DIN = 48
DHID = 128
NPAD = 100352
NLOC = NPAD // NCORES      # 12544
NB = NLOC // P             # 98
CHUNK = NPAD // 4          # 25088
NCLS = 8
GMAX = 1024                # dma_gather / dma_scatter_add index limit
GCOLS = GMAX // P          # 8 columns per gather instruction
TILE_COLS = 32             # bulk G-tile columns (4 gather instrs)
NRANGE = 4                 # mi accumulator ranges for MLP pipelining
QUANT_LAMBDA = 2           # DP grouping: column-cost of an extra fold group
COST_SLOT = 1.45            # rel cost of one bulk slot (Pool+DMA+DVE)
COST_OVF = 3.0             # rel cost of one overflow edge

_PROG_CACHE: dict = {}


# --------------------------------------------------------------------------
# host prep
# --------------------------------------------------------------------------

def _class_assign(dst, src):
    """2-choice greedy per-destination class balancing. Returns cls [E]."""
    E = len(dst)
    cA = src // CHUNK
    cB = 4 + (src % 4)
    sortp = np.argsort(dst, kind="stable")
    degd = np.bincount(dst, minlength=NPAD)
    first = np.cumsum(degd) - degd
    srank_d = np.arange(E) - first[dst[sortp]]
    loads = np.zeros((NPAD, NCLS), np.int32)
    cls = np.empty(E, np.int64)
    maxr = int(srank_d.max()) if E else 0
    for r in range(maxr + 1):
        sel = sortp[srank_d == r]
        if len(sel) == 0:
            continue
        d = dst[sel]
        la = loads[d, cA[sel]]
        lb = loads[d, cB[sel]]
        pick_b = lb < la
        c = np.where(pick_b, cB[sel], cA[sel])
        loads[d, c] += 1
        cls[sel] = c
    return cls


def _quantize_groups(T_col, cuts):
    """DP: partition blocks into contiguous groups (respecting forced cuts),
    pad T to group max. Returns [(b0, nbs, Tq)] with Tq>0 only."""
    groups = []
    for ci in range(len(cuts) - 1):
        lo, hi = cuts[ci], cuts[ci + 1]
        n = hi - lo
        if n <= 0:
            continue
        seg = T_col[lo:hi]
        INF = 1 << 60
        best = [INF] * (n + 1)
        prev = [0] * (n + 1)
        best[0] = 0
        for j in range(1, n + 1):
            m = 0
            ssum = 0
            for i in range(j - 1, -1, -1):
                if seg[i] > m:
                    m = seg[i]
                ssum += seg[i]
                cost = best[i] + (m * (j - i) - ssum) + QUANT_LAMBDA
                if cost < best[j]:
                    best[j] = cost
                    prev[j] = i
        bounds = []
        j = n
        while j > 0:
            bounds.append((prev[j], j))
            j = prev[j]
        for i, j in reversed(bounds):
            Tq = int(seg[i:j].max())
            if Tq > 0:
                groups.append((lo + i, j - i, Tq))
    return groups


def _wrap_idx(vals16, pos):
    """Scatter int16 values into the wrapped-by-16, 8x-replicated layout.
    vals16 [n], pos [n] list positions. Returns writes for a [16, L] array:
    (rows, cols)."""
    return (pos % 16).astype(np.int64), pos // 16


def _build_direction(dst, src, w, deg, core, ranges):
    """Full per-direction layout: bulk bands + overflow lists."""
    loc = np.empty(NPAD, np.int64)
    nodes_by_core = []
    for k in range(NCORES):
        nodes_k = np.where(core == k)[0]
        lk = nodes_k[np.argsort(-deg[nodes_k], kind="stable")]
        loc[lk] = np.arange(NLOC)
        nodes_by_core.append(lk)
    blk = loc // P
    part = loc % P

    cls = _class_assign(dst, src)
    E = len(dst)

    # per-edge rank within (dest, class)
    key = dst * NCLS + cls
    sortp = np.argsort(key, kind="stable")
    ks = key[sortp]
    cnt = np.bincount(ks, minlength=NPAD * NCLS)
    firstk = np.cumsum(cnt) - cnt
    srank = np.empty(E, np.int64)
    srank[sortp] = np.arange(E) - firstk[ks]

    loads = cnt.reshape(NPAD, NCLS)

    # T* per (block, class): minimize 128*T*COST_SLOT + unified_ovf(T)*COST_OVF
    maxl = int(loads.max())
    # histogram of per-(core, block, class) dest-loads
    hist = np.zeros((NCORES, NB, NCLS, maxl + 1), np.int64)
    nodes = np.arange(NPAD)
    for c in range(NCLS):
        np.add.at(hist, (core[nodes], blk[nodes], c, loads[nodes, c]), 1)
    # abovec[..., lv] = #dests with load > lv ; ovf(T) = sum_{lv>=T} abovec[lv]
    cum = hist.cumsum(axis=3)
    total = cum[..., -1:]
    abovec = (total - cum)  # [..., lv] = #dests with load > lv
    suf = abovec[..., ::-1].cumsum(axis=3)[..., ::-1]  # per-core ovf at T=lv
    ovf_unified = suf.max(axis=0)  # [NB, NCLS, maxl+1]
    Tcost = (128 * np.arange(maxl + 1)[None, None, :] * COST_SLOT
             + ovf_unified * COST_OVF)
    Tstar = Tcost.argmin(axis=2)  # [NB, NCLS]

    cuts = sorted(set([0, NB] + list(ranges)))
    groups_per_cls = [_quantize_groups(Tstar[:, c], cuts) for c in range(NCLS)]
    Tq = np.zeros((NB, NCLS), np.int64)
    for c in range(NCLS):
        for b0, nbs, t in groups_per_cls[c]:
            Tq[b0 : b0 + nbs, c] = t

    # ---- bulk tile packing ----
    # Per class: one continuous column run over all its groups (block order),
    # split into tiles of TILE_COLS; gather windows stay full except at each
    # class's tail. Tiles are interleaved across classes by block progress so
    # mi ranges complete roughly in order.
    cls_tiles = {c: [] for c in range(NCLS)}
    colstart = np.full((NB, NCLS), -1, np.int64)
    col = 0
    for c in range(NCLS):
        runs = []
        for b0, nbs, t in groups_per_cls[c]:
            nmax = max(1, TILE_COLS // t)
            j = 0
            while j < nbs:
                nn = min(nmax, nbs - j)
                runs.append((b0 + j, nn, t))
                j += nn
        cur = None
        for b0, nbs, t in runs:
            need = nbs * t
            if cur is None or cur["cols"] + need > TILE_COLS:
                if cur is not None:
                    cls_tiles[c].append(cur)
                cur = dict(cls=c, col0=col, cols=0, entries=[])
            cur["entries"].append((t, b0, nbs, cur["cols"]))
            for i in range(nbs):
                colstart[b0 + i, c] = col + i * t
            cur["cols"] += need
            col += need
        if cur is not None:
            cls_tiles[c].append(cur)
    CT = col
    # interleave tiles by first-block progress
    tiles = sorted(
        (t for c in range(NCLS) for t in cls_tiles[c]),
        key=lambda t: (t["entries"][0][1], t["cls"]),
    )

    percol_col0 = np.zeros(max(CT, 1), np.int64)
    for t in tiles:
        percol_col0[t["col0"] : t["col0"] + t["cols"]] = t["col0"]

    is_bulk = srank < Tq[blk[dst], cls]

    # class-local index values
    val = np.where(cls < 4, src - (cls * CHUNK), src // 4).astype(np.int16)

    eb = np.where(is_bulk)[0]
    d = dst[eb]
    k_e = core[d]
    b_e = blk[d]
    p_e = part[d]
    colg = colstart[b_e, cls[eb]] + srank[eb]
    col0 = percol_col0[colg]
    g_t = (colg - col0) * P + p_e
    ic = col0 * 8 + g_t // 16
    ir = g_t % 16

    idx16 = np.zeros((NCORES, 16, max(CT, 1) * 8), np.int16)
    idx16[k_e, ir, ic] = val[eb]
    idx_arr = np.tile(idx16, (1, 8, 1))
    w_arr = np.zeros((NCORES, P, max(CT, 1)), np.float16)
    w_arr[k_e, p_e, colg] = w[eb].astype(np.float16)

    # ---- overflow lists ----
    # order: (rank', class, dest); windows of GMAX cut at rank boundaries
    eo = np.where(~is_bulk)[0]
    # global per-dest overflow rank (across classes): scatter windows slice by
    # rank, so a dest must appear at most once per rank block
    osort0 = np.argsort(dst[eo], kind="stable")
    dso = dst[eo][osort0]
    dcnt = np.bincount(dso, minlength=NPAD)
    dfirst = np.cumsum(dcnt) - dcnt
    rank2 = np.empty(len(eo), np.int64)
    rank2[osort0] = np.arange(len(eo)) - dfirst[dso]
    # per-core list; unified layout = same windows/segments across cores, so
    # build per-core orderings with shared per-(rank, class) segment sizes
    segcnt = np.zeros((NCORES, int(rank2.max()) + 1 if len(eo) else 1, NCLS),
                      np.int64)
    if len(eo):
        np.add.at(segcnt, (core[dst[eo]], rank2, cls[eo]), 1)
    useg = segcnt.max(axis=0)  # unified per-(rank, class) segment length
    # pad each segment to a 128 multiple: every gather instruction restarts
    # its list at partition 0, so segments must start 128-aligned
    useg = -(-useg // P) * P
    NRK = useg.shape[0]
    seg_off = np.zeros((NRK, NCLS), np.int64)
    posn = 0
    ov_windows = []   # pos0 per GMAX window
    ov_gathers = []   # (cls, pos0, n) per gather instruction (<= GMAX, class-pure)
    for r in range(NRK):
        for c in range(NCLS):
            n = int(useg[r, c])
            if n == 0:
                continue
            seg_off[r, c] = posn
            s = 0
            while s < n:
                woff = (posn + s) % GMAX
                take = min(n - s, GMAX - woff)
                ov_gathers.append((c, posn + s, take))
                s += take
            posn += n
        posn = -(-posn // GMAX) * GMAX  # pad rank block to window multiple
    OVT = posn // P  # overflow columns
    for wpos in range(0, posn, GMAX):
        ov_windows.append(wpos)

    ov_idx16 = np.zeros((NCORES, 16, max(OVT, 1) * 8), np.int16)
    ov_sidx16 = np.full((NCORES, 16, max(OVT, 1) * 8), NLOC, np.int16)
    ov_w = np.zeros((NCORES, P, max(OVT, 1)), np.float16)
    if len(eo):
        # per-core position within segment: stable order by (core,rank,cls)
        okey = (core[dst[eo]] * NRK + rank2) * NCLS + cls[eo]
        osort = np.argsort(okey, kind="stable")
        oks = okey[osort]
        ocnt = np.bincount(oks, minlength=NCORES * NRK * NCLS)
        ofirst = np.cumsum(ocnt) - ocnt
        opos = np.empty(len(eo), np.int64)
        opos[osort] = np.arange(len(eo)) - ofirst[oks]
        e = eo
        posg = seg_off[rank2, cls[e]] + opos  # list position
        kk = core[dst[e]]
        ov_idx16[kk, posg % 16, (posg // 16)] = val[e]
        ov_sidx16[kk, posg % 16, (posg // 16)] = loc[dst[e]].astype(np.int16)
        ov_w[kk, posg % P, posg // P] = w[e].astype(np.float16)
    ov_idx = np.tile(ov_idx16, (1, 8, 1))
    ov_sidx = np.tile(ov_sidx16, (1, 8, 1))

    return dict(
        loc=loc, blk=blk, part=part, nodes_by_core=nodes_by_core,
        tiles=tiles, CT=CT, idx_arr=idx_arr, w_arr=w_arr, Tq=Tq,
        OVT=OVT, ov_idx=ov_idx, ov_sidx=ov_sidx, ov_w=ov_w,
        ov_windows=ov_windows, ov_gathers=ov_gathers,
    )


def _host_prep(x, edge_index, edge_attr):
    N = x.shape[0]
    row = np.asarray(edge_index[0]).astype(np.int64)
    col = np.asarray(edge_index[1]).astype(np.int64)
    w = np.asarray(edge_attr, dtype=np.float32).reshape(-1)

    deg_in = np.bincount(col, minlength=NPAD)
    deg_out = np.bincount(row, minlength=NPAD)
    order = np.argsort(-(deg_in + deg_out), kind="stable")
    rank = np.empty(NPAD, np.int64)
    rank[order] = np.arange(NPAD)
    core = (rank % NCORES).astype(np.int64)

    ranges = [30, 60, 90]

    dmi = _build_direction(col, row, w, deg_in, core, ranges)
    dmo = _build_direction(row, col, w, deg_out, core, [])

    xf = np.zeros((NPAD, DIN), np.float32)
    xf[:N] = np.asarray(x, np.float32)
    x16 = xf.astype(np.float16)
    tabs = np.zeros((NCLS, CHUNK, 128), np.float16)
    for c in range(4):
        tabs[c, :, :DIN] = x16[c * CHUNK : (c + 1) * CHUNK]
    for r in range(4):
        tabs[4 + r, :, :DIN] = x16[r::4]

    x_own = np.zeros((NCORES, P, NB * DIN), np.float16)
    for k in range(NCORES):
        lk = dmi["nodes_by_core"][k]
        xv = x16[lk]
        x_own[k] = (
            xv.reshape(NB, P, DIN).transpose(1, 0, 2).reshape(P, NB * DIN)
        )

    realign = np.zeros((NCORES, 16, NLOC // 16), np.int16)
    g = np.arange(NLOC)
    for k in range(NCORES):
        lk = dmi["nodes_by_core"][k]
        vals = dmo["loc"][lk].astype(np.int16)
        realign[k, g % 16, g // 16] = vals
    realign = np.tile(realign, (1, 8, 1))

    return dict(
        N=N, core=core, dmi=dmi, dmo=dmo, tabs=tabs, x_own=x_own,
        realign=realign,
    )


# --------------------------------------------------------------------------
# numpy emulation (layout validation)
# --------------------------------------------------------------------------

def _emulate_agg(meta, direction):
    """Emulate both tiers -> acc [NCORES, 128, NB*48] fp32 in local order."""
    d = meta[direction]
    tabs = meta["tabs"]
    CT = d["CT"]
    acc = np.zeros((NCORES, P, NB, DIN), np.float32)
    colcls = np.zeros(max(CT, 1), np.int64)
    colblk = np.zeros(max(CT, 1), np.int64)
    col0a = np.zeros(max(CT, 1), np.int64)
    for t in d["tiles"]:
        col0a[t["col0"] : t["col0"] + t["cols"]] = t["col0"]
        for s, b0, nbs, cbase in t["entries"]:
            for i in range(nbs):
                c0 = t["col0"] + cbase + i * s
                colcls[c0 : c0 + s] = t["cls"]
                colblk[c0 : c0 + s] = b0 + i
    for k in range(NCORES):
        if CT:
            idx = d["idx_arr"][k]
            wv = d["w_arr"][k]
            cols = np.arange(CT)
            pp = np.arange(P)
            g_t = (cols[None, :] - col0a[None, :]) * P + pp[:, None]
            ic = col0a[None, :] * 8 + g_t // 16
            ir = g_t % 16
            vals = idx[ir, ic].astype(np.int64)
            gathered = tabs[colcls[None, :].repeat(P, 0), vals][:, :, :DIN]
            prod = gathered.astype(np.float16) * wv[:, :, None]
            np.add.at(acc[k], (slice(None), colblk), prod.astype(np.float32))
        # overflow
        OVT = d["OVT"]
        if OVT:
            oi = d["ov_idx"][k]
            os_ = d["ov_sidx"][k]
            ow = d["ov_w"][k]
            # reconstruct per-position
            ocls = np.zeros(OVT * P, np.int64)
            for c, pos0, n in d["ov_gathers"]:
                ocls[pos0 : pos0 + n] = c
            g = np.arange(OVT * P)
            vals = oi[g % 16, g // 16].astype(np.int64)
            sidx = os_[g % 16, g // 16].astype(np.int64)
            wvals = ow[g % P, g // P].astype(np.float16)
            gath = tabs[ocls, vals][:, :DIN].astype(np.float16)
            prod = (gath * wvals[:, None]).astype(np.float32)
            sel = sidx < NLOC
            tgt = sidx[sel]
            np.add.at(acc[k], (tgt % P, tgt // P), prod[sel])
    return acc.reshape(NCORES, P, NB * DIN)


def _emulate(meta, W1, b1, W2, b2):
    mi = _emulate_agg(meta, "dmi")
    mo = _emulate_agg(meta, "dmo")
    out = np.zeros((NPAD, DHID), np.float32)
    for k in range(NCORES):
        lk_i = meta["dmi"]["nodes_by_core"][k]
        lk_o = meta["dmo"]["nodes_by_core"][k]
        mi_k = mi[k].reshape(P, NB, DIN).transpose(1, 0, 2).reshape(NLOC, DIN)
        mo_k = mo[k].reshape(P, NB, DIN).transpose(1, 0, 2).reshape(NLOC, DIN)
        mo_full = np.zeros((NLOC, DIN), np.float32)
        mo_full[:] = mo_k  # mo-local order
        mo2_k = mo_full[meta["dmo"]["loc"][lk_i]]
        x_k = meta["x_own"][k].reshape(P, NB, DIN).transpose(1, 0, 2).reshape(
            NLOC, DIN
        ).astype(np.float32)
        M = np.concatenate([mi_k, mo2_k, x_k], axis=1)
        h = np.tanh(M @ W1.T + b1)
        out[lk_i] = np.tanh(h @ W2.T + b2)
    return out[: meta["N"]]


# --------------------------------------------------------------------------
# device program
# --------------------------------------------------------------------------

def _dma_gather96(gp, mybir, out_ap, in_ap, idxs_ap, num_idxs):
    from concourse.bass import exact_div

    elem_step = in_ap.ap[0][0]
    stride_bytes = elem_step * mybir.dt.size(in_ap.dtype)
    return gp.add_instruction(
        mybir.InstDMAGatherAnt(
            name=gp.bass.get_next_instruction_name(),
            ins=[
                *gp.lower_ap_dma(in_ap, for_custom_bir_dma=True),
                gp.lower_ap(idxs_ap),
                gp.lower_val_access(gp.to_reg(num_idxs)),
            ],
            outs=[gp.lower_ap(out_ap)],
            transpose=False,
            num_idxs=num_idxs,
            elem_size=DIN,
            stride_bytes_256=exact_div(stride_bytes, 256),
            gen_mode=0,
            single_packet=True,
            queue_num=0,
            sbuf_tokens_per_rank=0,
            sbuf_free_dim_per_rank=0,
            sbuf_free_dim_pad_per_rank=0,
            sbuf_byte_offset=0,
        )
    )


def _build_program(meta):
    import concourse.bacc as bacc
    import concourse.bass as bass
    import concourse.mybir as mybir
    import concourse.tile as tile
    from concourse.masks import make_identity

    f32 = mybir.dt.float32
    f16 = mybir.dt.float16
    i16 = mybir.dt.int16

    dmi, dmo = meta["dmi"], meta["dmo"]
    CTI, CTO = max(dmi["CT"], 1), max(dmo["CT"], 1)
    OVI, OVO = max(dmi["OVT"], 1), max(dmo["OVT"], 1)

    nc = bacc.Bacc(
        "TRN2",
        target_bir_lowering=False,
        debug=False,
        num_devices=NCORES,
        dynamic_dma_scratch_size=65536,
    )

    tabs_d = [
        nc.dram_tensor(f"tab{c}", [CHUNK, 128], f16, kind="ExternalInput")
        for c in range(NCLS)
    ]
    idx_mi_d = nc.dram_tensor("idx_mi", [P, CTI * 8], i16, kind="ExternalInput")
    idx_mo_d = nc.dram_tensor("idx_mo", [P, CTO * 8], i16, kind="ExternalInput")
    w_mi_d = nc.dram_tensor("w_mi", [P, CTI], f16, kind="ExternalInput")
    w_mo_d = nc.dram_tensor("w_mo", [P, CTO], f16, kind="ExternalInput")
    ovi_idx_d = nc.dram_tensor("ovi_idx", [P, OVI * 8], i16, kind="ExternalInput")
    ovi_sidx_d = nc.dram_tensor("ovi_sidx", [P, OVI * 8], i16, kind="ExternalInput")
    ovi_w_d = nc.dram_tensor("ovi_w", [P, OVI], f16, kind="ExternalInput")
    ovo_idx_d = nc.dram_tensor("ovo_idx", [P, OVO * 8], i16, kind="ExternalInput")
    ovo_sidx_d = nc.dram_tensor("ovo_sidx", [P, OVO * 8], i16, kind="ExternalInput")
    ovo_w_d = nc.dram_tensor("ovo_w", [P, OVO], f16, kind="ExternalInput")
    x_own_d = nc.dram_tensor("x_own", [P, NB * DIN], f16, kind="ExternalInput")
    realign_d = nc.dram_tensor("realign", [P, NLOC // 16], i16, kind="ExternalInput")
    mi_acc = nc.dram_tensor("mi_acc", [NLOC + P, 128], f16, kind="Internal")
    mo_acc = nc.dram_tensor("mo_acc", [NLOC + P, 128], f16, kind="Internal")
    w1ta_d = nc.dram_tensor("w1ta", [DIN, DHID], f16, kind="ExternalInput")
    w1tb_d = nc.dram_tensor("w1tb", [DIN, DHID], f16, kind="ExternalInput")
    w1tc_d = nc.dram_tensor("w1tc", [DIN, DHID], f16, kind="ExternalInput")
    w2t_d = nc.dram_tensor("w2t", [DHID, DHID], f16, kind="ExternalInput")
    b1_d = nc.dram_tensor("b1", [DHID, 1], f32, kind="ExternalInput")
    b2_d = nc.dram_tensor("b2", [DHID, 1], f32, kind="ExternalInput")
    out_t = nc.dram_tensor("out_t", [P, NLOC], f16, kind="ExternalOutput")

    rbounds = [0, 30, 60, 90, NB]

    with tile.TileContext(nc) as tc:
        with (
            tc.tile_pool(name="const", bufs=1) as const,
            tc.tile_pool(name="gidx", bufs=4) as gidx,
            tc.tile_pool(name="gpool", bufs=4) as gpool,
            tc.tile_pool(name="mlp", bufs=4) as mlp,
            tc.tile_pool(name="ost", bufs=2) as ostp,
            tc.tile_pool(name="psT", bufs=3, space="PSUM") as psT,
            tc.tile_pool(name="psH", bufs=2, space="PSUM") as psH,
        ):
            w_mi_sb = const.tile([P, CTI], f16)
            nc.sync.dma_start(w_mi_sb[:], w_mi_d[:])
            w_mo_sb = const.tile([P, CTO], f16)
            nc.sync.dma_start(w_mo_sb[:], w_mo_d[:])
            ovi_w_sb = const.tile([P, OVI], f16)
            nc.sync.dma_start(ovi_w_sb[:], ovi_w_d[:])
            ovo_w_sb = const.tile([P, OVO], f16)
            nc.sync.dma_start(ovo_w_sb[:], ovo_w_d[:])
            realign_sb = const.tile([P, NLOC // 16], i16)
            nc.sync.dma_start(realign_sb[:], realign_d[:])
            w1ta_sb = const.tile([DIN, DHID], f16)
            nc.sync.dma_start(w1ta_sb[:], w1ta_d[:])
            w1tb_sb = const.tile([DIN, DHID], f16)
            nc.sync.dma_start(w1tb_sb[:], w1tb_d[:])
            w1tc_sb = const.tile([DIN, DHID], f16)
            nc.sync.dma_start(w1tc_sb[:], w1tc_d[:])
            w2t_sb = const.tile([DHID, DHID], f16)
            nc.sync.dma_start(w2t_sb[:], w2t_d[:])
            b1_sb = const.tile([DHID, 1], f32)
            nc.sync.dma_start(b1_sb[:], b1_d[:])
            b2_sb = const.tile([DHID, 1], f32)
            nc.sync.dma_start(b2_sb[:], b2_d[:])
            ident = const.tile([P, P], f16)
            make_identity(nc, ident[:])
            xo_sb = const.tile([P, NB * DIN], f16)
            nc.sync.dma_start(xo_sb[:], x_own_d[:])

            mo_sb = const.tile([P, NB * DIN], f16)
            mo2_sb = const.tile([P, NB * DIN], f16)
            mi_rs = []
            for ri in range(NRANGE):
                mi_ri = const.tile([P, (rbounds[ri + 1] - rbounds[ri]) * DIN],
                                   f16, name=f"mi_r{ri}")
                mi_rs.append(mi_ri)
                nc.vector.memset(mi_ri[:], 0.0)
            zz = const.tile([P, (NB + 1) * DIN], f16)
            nc.vector.memset(mo_sb[:], 0.0)
            nc.vector.memset(zz[:], 0.0)
            # zero DRAM accumulators (NLOC + dummy block rows)
            for acc_d in (mi_acc, mo_acc):
                nc.sync.dma_start(
                    acc_d[:, 0:DIN].rearrange("(b p) f -> p b f", p=P),
                    zz[:].rearrange("p (b f) -> p b f", f=DIN),
                )

            def emit_interleaved(dirmeta, idx_d, w_sb, acc_of,
                                 oidx_d, osidx_d, ow_sb, acc_d,
                                 only_range=None):
                tiles = [t for t in dirmeta["tiles"]
                         if only_range is None
                         or only_range[0] <= t["entries"][0][1] < only_range[1]]
                wins = list(dirmeta["ov_windows"]) if only_range is None or \
                    only_range[0] == 0 else []
                step = max(1, (len(tiles) + len(wins)) // max(len(wins), 1)) \
                    if wins else 1 << 30
                wi = 0
                for i, t in enumerate(tiles):
                    emit_one_bulk(dirmeta, t, idx_d, w_sb, acc_of)
                    if wi < len(wins) and i % step == step - 1:
                        emit_overflow(dirmeta, oidx_d, osidx_d, ow_sb, acc_d,
                                      [wins[wi]])
                        wi += 1
                if wi < len(wins):
                    emit_overflow(dirmeta, oidx_d, osidx_d, ow_sb, acc_d,
                                  wins[wi:])

            def emit_bulk(dirmeta, idx_d, w_sb, acc_of, only_range=None):
                rlo, rhi = (0, NB) if only_range is None else only_range
                for t in dirmeta["tiles"]:
                    b_first = t["entries"][0][1]
                    if not (rlo <= b_first < rhi):
                        continue
                    emit_one_bulk(dirmeta, t, idx_d, w_sb, acc_of)

            def emit_one_bulk(dirmeta, t, idx_d, w_sb, acc_of):
                    cols = t["cols"]
                    c = t["cls"]
                    col0 = t["col0"]
                    idx_sb = gidx.tile([P, TILE_COLS * 8], i16, tag="gi")
                    nc.sync.dma_start(
                        idx_sb[:, : cols * 8],
                        idx_d[:, col0 * 8 : (col0 + cols) * 8],
                    )
                    G = gpool.tile([P, TILE_COLS * DIN], f16, tag="G")
                    for q0 in range(0, cols, GCOLS):
                        qn = min(GCOLS, cols - q0)
                        _dma_gather96(
                            nc.gpsimd, mybir,
                            out_ap=G[:, q0 * DIN : (q0 + qn) * DIN].rearrange(
                                "p (c f) -> p c f", f=DIN
                            ),
                            in_ap=tabs_d[c][:, 0:DIN],
                            idxs_ap=idx_sb[:, q0 * 8 : (q0 + qn) * 8],
                            num_idxs=qn * P,
                        )
                    g3 = G[:, : cols * DIN].rearrange("p (c f) -> p c f", f=DIN)
                    wv = w_sb[:, col0 : col0 + cols]
                    wb = bass.AP(
                        wv.tensor,
                        wv.offset,
                        [list(wv.ap[0]), list(wv.ap[1]), [0, DIN]],
                    )
                    nc.vector.tensor_tensor(
                        out=g3, in0=g3, in1=wb, op=mybir.AluOpType.mult
                    )
                    for s, b0, nbs, cbase in t["entries"]:
                        gg = G[
                            :, cbase * DIN : (cbase + nbs * s) * DIN
                        ].rearrange("p (b s f) -> p b s f", s=s, f=DIN)
                        ss = s
                        while ss > 1:
                            half = ss // 2
                            hi0 = ss - half
                            nc.vector.tensor_tensor(
                                out=gg[:, :, 0:half, :],
                                in0=gg[:, :, 0:half, :],
                                in1=gg[:, :, hi0:ss, :],
                                op=mybir.AluOpType.add,
                            )
                            ss = hi0
                        acc_sb, boff = acc_of(b0)
                        accv = acc_sb[
                            :, (b0 - boff) * DIN : (b0 - boff + nbs) * DIN
                        ].rearrange("p (b f) -> p b f", f=DIN)
                        nc.vector.tensor_tensor(
                            out=accv, in0=accv, in1=gg[:, :, 0, :],
                            op=mybir.AluOpType.add,
                        )

            def emit_overflow(dirmeta, oidx_d, osidx_d, ow_sb, acc_d,
                              subset=None):
                gathers = dirmeta["ov_gathers"]
                windows = dirmeta["ov_windows"] if subset is None else subset
                for wpos in windows:
                    gi = 0
                    while gi < len(gathers) and gathers[gi][1] < wpos:
                        gi += 1
                    idx_sb = gidx.tile([P, GCOLS * 8], i16, tag="oi")
                    nc.sync.dma_start(
                        idx_sb[:],
                        oidx_d[:, (wpos // 16) : (wpos // 16) + GCOLS * 8],
                    )
                    sidx_sb = gidx.tile([P, GCOLS * 8], i16, tag="os")
                    nc.sync.dma_start(
                        sidx_sb[:],
                        osidx_d[:, (wpos // 16) : (wpos // 16) + GCOLS * 8],
                    )
                    G = gpool.tile([P, GCOLS * DIN], f16, tag="G")
                    while gi < len(gathers) and gathers[gi][1] < wpos + GMAX:
                        c, pos0, n = gathers[gi]
                        lo = pos0 - wpos
                        _dma_gather96(
                            nc.gpsimd, mybir,
                            out_ap=G[
                                :, (lo // P) * DIN : ((lo + n) // P) * DIN
                            ].rearrange("p (c f) -> p c f", f=DIN),
                            in_ap=tabs_d[c][:, 0:DIN],
                            idxs_ap=idx_sb[:, lo // 16 : (lo + n) // 16],
                            num_idxs=n,
                        )
                        gi += 1
                    g3 = G[:].rearrange("p (c f) -> p c f", f=DIN)
                    wv = ow_sb[:, wpos // P : wpos // P + GCOLS]
                    wb = bass.AP(
                        wv.tensor,
                        wv.offset,
                        [list(wv.ap[0]), list(wv.ap[1]), [0, DIN]],
                    )
                    nc.vector.tensor_tensor(
                        out=g3, in0=g3, in1=wb, op=mybir.AluOpType.mult
                    )
                    nc.gpsimd.dma_scatter_add(
                        out_ap=acc_d[:, 0:DIN],
                        in_ap=g3,
                        idxs_ap=sidx_sb[:],
                        num_idxs=GMAX,
                        num_idxs_reg=GMAX,
                        elem_size=DIN,
                        elem_step=128,
                    )

            def acc_mo(b0):
                return mo_sb, 0

            def acc_mi(b0):
                for ri in range(NRANGE):
                    if b0 < rbounds[ri + 1]:
                        return mi_rs[ri], rbounds[ri]
                raise AssertionError(b0)

            # ---- mo: bulk with overflow windows interleaved ----
            emit_interleaved(dmo, idx_mo_d, w_mo_sb, acc_mo,
                             ovo_idx_d, ovo_sidx_d, ovo_w_sb, mo_acc)
            # merge: mo_sb += mo_acc; write back
            tmp = const.tile([P, NB * DIN], f16)
            nc.sync.dma_start(
                tmp[:].rearrange("p (b f) -> p b f", f=DIN),
                mo_acc[0:NLOC, 0:DIN].rearrange("(b p) f -> p b f", p=P),
            )
            nc.vector.tensor_tensor(
                out=mo_sb[:], in0=mo_sb[:], in1=tmp[:], op=mybir.AluOpType.add
            )
            nc.sync.dma_start(
                mo_acc[0:NLOC, 0:DIN].rearrange("(b p) f -> p b f", p=P),
                mo_sb[:].rearrange("p (b f) -> p b f", f=DIN),
            )

            def emit_mlp(rlo, rhi):
                OG = 4
                for b0 in range(rlo, rhi, OG):
                    og = min(OG, rhi - b0)
                    os_ = ostp.tile([P, OG * P], f16, tag="oo")
                    for j in range(og):
                        b = b0 + j
                        hp = psH.tile([P, P], f32, tag="hp")
                        mi_t, mi_b0 = acc_mi(b)
                        for q, (src_sb, bb, w1q) in enumerate((
                            (mi_t, b - mi_b0, w1ta_sb),
                            (mo2_sb, b, w1tb_sb),
                            (xo_sb, b, w1tc_sb),
                        )):
                            pA = psT.tile([DIN, P], f16, tag="pA")
                            nc.tensor.transpose(
                                pA[:], src_sb[:, bb * DIN : (bb + 1) * DIN],
                                ident[:],
                            )
                            mt = mlp.tile([DIN, P], f16, tag="mt")
                            nc.vector.tensor_copy(out=mt[:], in_=pA[:])
                            nc.tensor.matmul(
                                hp[:], w1q[:], mt[:],
                                start=(q == 0), stop=(q == 2),
                            )
                        hs = mlp.tile([P, P], f16, tag="hs")
                        nc.scalar.activation(
                            hs[:], hp[:],
                            mybir.ActivationFunctionType.Tanh,
                            bias=b1_sb[:], scale=1.0,
                        )
                        op_ = psH.tile([P, P], f32, tag="op")
                        nc.tensor.matmul(
                            op_[:], w2t_sb[:], hs[:], start=True, stop=True
                        )
                        nc.scalar.activation(
                            os_[:, j * P : (j + 1) * P], op_[:],
                            mybir.ActivationFunctionType.Tanh,
                            bias=b2_sb[:], scale=1.0,
                        )
                    nc.sync.dma_start(
                        out_t[:, b0 * P : (b0 + og) * P], os_[:, : og * P]
                    )

            realigned = False
            for ri in range(NRANGE):
                rlo, rhi = rbounds[ri], rbounds[ri + 1]
                if ri == 0:
                    emit_interleaved(dmi, idx_mi_d, w_mi_sb, acc_mi,
                                     ovi_idx_d, ovi_sidx_d, ovi_w_sb, mi_acc,
                                     (rlo, rhi))
                else:
                    emit_bulk(dmi, idx_mi_d, w_mi_sb, acc_mi, (rlo, rhi))
                if not realigned:
                    # realign mo to mi-local order (after some mi bulk so the
                    # Pool queue is not head-of-line blocked on the mo merge)
                    for g0 in range(0, NLOC, GMAX):
                        gn = min(GMAX, NLOC - g0)
                        _dma_gather96(
                            nc.gpsimd, mybir,
                            out_ap=mo2_sb[
                                :, (g0 // P) * DIN : ((g0 + gn) // P) * DIN
                            ].rearrange("p (b f) -> p b f", f=DIN),
                            in_ap=mo_acc[:, 0:DIN],
                            idxs_ap=realign_sb[:, g0 // 16 : (g0 + gn) // 16],
                            num_idxs=gn,
                        )
                    realigned = True
                # merge overflow acc for this range
                tmpr_full = gpool.tile([P, 30 * DIN], f16, tag="mr")
                tmpr = tmpr_full[:, : (rhi - rlo) * DIN]
                nc.sync.dma_start(
                    tmpr[:].rearrange("p (b f) -> p b f", f=DIN),
                    mi_acc[rlo * P : rhi * P, 0:DIN].rearrange(
                        "(b p) f -> p b f", p=P
                    ),
                )
                nc.vector.tensor_tensor(
                    out=mi_rs[ri][:],
                    in0=mi_rs[ri][:],
                    in1=tmpr[:],
                    op=mybir.AluOpType.add,
                )
                emit_mlp(rlo, rhi)

    nc.compile()
    return nc


# --------------------------------------------------------------------------
# entry point
# --------------------------------------------------------------------------

def kernel(x, edge_index, edge_attr, W1, b1, W2, b2):
    x = np.asarray(x, np.float32)
    meta = _host_prep(x, edge_index, edge_attr)
    dmi, dmo = meta["dmi"], meta["dmo"]
    key = (meta["N"], dmi["CT"], dmo["CT"], dmi["OVT"], dmo["OVT"],
           tuple(t["col0"] for t in dmi["tiles"]),
           tuple(t["col0"] for t in dmo["tiles"]),
           tuple(dmi["ov_gathers"]), tuple(dmo["ov_gathers"]))
    if key not in _PROG_CACHE:
        _PROG_CACHE[key] = _build_program(meta)
    nc = _PROG_CACHE[key]

    W1 = np.asarray(W1, np.float32)
    W2 = np.asarray(W2, np.float32)
    b1v = np.asarray(b1, np.float32).reshape(DHID, 1)
    b2v = np.asarray(b2, np.float32).reshape(DHID, 1)
    w1t = np.ascontiguousarray(W1.T)
    w1ta = np.ascontiguousarray(w1t[:DIN]).astype(np.float16)
    w1tb = np.ascontiguousarray(w1t[DIN : 2 * DIN]).astype(np.float16)
    w1tc = np.ascontiguousarray(w1t[2 * DIN :]).astype(np.float16)
    w2t = np.ascontiguousarray(W2.T).astype(np.float16)

    in_maps = []
    for k in range(NCORES):
        m = {
            "idx_mi": dmi["idx_arr"][k],
            "idx_mo": dmo["idx_arr"][k],
            "w_mi": dmi["w_arr"][k],
            "w_mo": dmo["w_arr"][k],
            "ovi_idx": dmi["ov_idx"][k],
            "ovi_sidx": dmi["ov_sidx"][k],
            "ovi_w": dmi["ov_w"][k],
            "ovo_idx": dmo["ov_idx"][k],
            "ovo_sidx": dmo["ov_sidx"][k],
            "ovo_w": dmo["ov_w"][k],
            "x_own": meta["x_own"][k],
            "realign": meta["realign"][k],
            "w1ta": w1ta, "w1tb": w1tb, "w1tc": w1tc, "w2t": w2t,
            "b1": b1v, "b2": b2v,
        }
        for c in range(NCLS):
            m[f"tab{c}"] = meta["tabs"][c]
        in_maps.append(m)

    runner = _get_runner(nc)
    results = runner.run(in_maps)
    global _LAST
    _LAST = (nc, in_maps)

    out = np.empty((NPAD, DHID), np.float32)
    for k in range(NCORES):
        out[dmi["nodes_by_core"][k]] = results[k]["out_t"].T.astype(np.float32)
    return out[: meta["N"]]


_LAST = None
_RUNNER_CACHE: dict = {}


class _PjrtRunner:
    """Builds the shard_map-jitted NEFF executor once; supports repeated
    dispatches with device-resident inputs for timing."""

    def __init__(self, nc):
        import jax
        import jax.numpy as jnp
        import concourse.mybir as mybir
        from concourse import bass2jax
        from jax.sharding import Mesh, NamedSharding, PartitionSpec
        from jax.experimental.shard_map import shard_map

        bass2jax.install_neuronx_cc_hook()
        self.jax = jax
        self.jnp = jnp
        in_names: list[str] = []
        out_names: list[str] = []
        out_avals = []
        out_shapes = []
        partition_name = (
            nc.partition_id_tensor.name if nc.partition_id_tensor else None
        )
        for alloc in nc.m.functions[0].allocations:
            if not isinstance(alloc, mybir.MemoryLocationSet):
                continue
            name = alloc.memorylocations[0].name
            if alloc.kind == "ExternalInput":
                if name != partition_name:
                    in_names.append(name)
            elif alloc.kind == "ExternalOutput":
                shape = tuple(alloc.tensor_shape)
                dtype = mybir.dt.np(alloc.dtype)
                out_names.append(name)
                out_avals.append(jax.core.ShapedArray(shape, dtype))
                out_shapes.append((shape, dtype))
        self.in_names = in_names
        self.out_names = out_names
        self.out_shapes = out_shapes
        n_params = len(in_names)
        n_outs = len(out_names)
        all_names = in_names + out_names
        if partition_name is not None:
            all_names = all_names + [partition_name]

        def _body(*args):
            operands = list(args)
            if partition_name is not None:
                operands.append(bass2jax.partition_id_tensor())
            outs = bass2jax._bass_exec_p.bind(
                *operands,
                out_avals=tuple(out_avals),
                in_names=tuple(all_names),
                out_names=tuple(out_names),
                lowering_input_output_aliases=(),
                sim_require_finite=True,
                sim_require_nnan=True,
                nc=nc,
            )
            return tuple(outs)

        devices = jax.devices()[:NCORES]
        self.mesh = Mesh(np.asarray(devices), ("core",))
        spec = PartitionSpec("core")
        self.sharding = NamedSharding(self.mesh, spec)
        self.sharded = jax.jit(
            shard_map(
                _body,
                mesh=self.mesh,
                in_specs=(spec,) * (n_params + n_outs),
                out_specs=(spec,) * n_outs,
                check_rep=False,
            ),
            donate_argnums=tuple(range(n_params, n_params + n_outs)),
            keep_unused=True,
        )

        def _mk_zeros():
            return tuple(
                jnp.zeros((NCORES * s[0], *s[1:]), d) for s, d in out_shapes
            )

        self.zeros_fn = jax.jit(
            _mk_zeros, out_shardings=(self.sharding,) * n_outs
        )

    def _stage_inputs(self, in_maps):
        concat = [
            np.concatenate(
                [np.asarray(in_maps[c][n]) for c in range(NCORES)], axis=0
            )
            for n in self.in_names
        ]
        return [self.jax.device_put(a, self.sharding) for a in concat]

    def _dispatch(self, staged):
        zeros = self.zeros_fn()
        outs = self.sharded(*staged, *zeros)
        self.jax.block_until_ready(outs)
        return outs

    def run(self, in_maps):
        staged = self._stage_inputs(in_maps)
        outs = self._dispatch(staged)
        res = []
        for c in range(NCORES):
            m = {}
            for i, n in enumerate(self.out_names):
                s, d = self.out_shapes[i]
                m[n] = np.asarray(outs[i]).reshape(NCORES, *s)[c]
            res.append(m)
        return res

    def timed(self, in_maps, iters=10):
        import time

        staged = self._stage_inputs(in_maps)
        self._dispatch(staged)  # warm
        walls = []
        for _ in range(iters):
            zeros = self.zeros_fn()
            self.jax.block_until_ready(zeros)
            t0 = time.perf_counter()
            outs = self.sharded(*staged, *zeros)
            self.jax.block_until_ready(outs)
            walls.append(time.perf_counter() - t0)
        tiny = self.jax.device_put(
            np.zeros((NCORES, 8), np.float32), self.sharding
        )
        base_fn = self.jax.jit(lambda a: a + 1.0)
        self.jax.block_until_ready(base_fn(tiny))
        bases = []
        for _ in range(iters):
            t0 = time.perf_counter()
            self.jax.block_until_ready(base_fn(tiny))
            bases.append(time.perf_counter() - t0)
        print(
            f"kernel walls min/med: {min(walls)*1e3:.2f}/"
            f"{np.median(walls)*1e3:.2f} ms; "
            f"baseline min/med: {min(bases)*1e3:.2f}/"
            f"{np.median(bases)*1e3:.2f} ms"
        )
        return max(float(np.median(walls) - np.median(bases)), 0.0) * 1e9


def _get_runner(nc):
    r = _RUNNER_CACHE.get(id(nc))
    if r is None:
        r = _PjrtRunner(nc)
        _RUNNER_CACHE[id(nc)] = r
    return r


def time_kernel(inputs=None, iters=8):
    assert _LAST is not None, "call kernel() first"
    nc, in_maps = _LAST
    return _get_runner(nc).timed(in_maps, iters=iters)


# revision 23
# speedup vs baseline: 8.3404x; 2.9573x over previous
"""Trainium2 Bass kernel for NodeNetworkG GNN message passing.

Algorithm (8 NeuronCores, SPMD, no collectives):
  - Nodes are sorted by total degree and dealt round-robin to 8 cores; each
    core owns ~1/8 of nodes and edges for both aggregation directions.
  - Per direction, destinations are packed into 128-node blocks (sorted by
    that direction's degree). Each edge is assigned to one of 8 "classes"
    (4 node-id range chunks + 4 mod-4 decimated copies) via 2-choice load
    balancing, so every gather instruction's int16 indices address a
    25088-row fp16 table (dma_gather is limited to 1024 indices/instruction).
  - Bulk tier: per (block, class) band with T slots per destination (T chosen
    per band to trade slot padding vs overflow, quantized over block groups).
    Batched dma_gather (96B rows from 256B-stride tables), DVE weight-multiply
    and uniform-T fold in fp16, accumulated into [128, nb*48] accumulators.
  - Overflow tier (edges beyond T): class-grouped dma_gather at full packing,
    weight-multiply, then dma_scatter_add into a DRAM accumulator (rank-sliced
    windows keep destinations unique per scatter instruction).
  - mo is realigned to mi-local node order via the DRAM accumulator and one
    dma_gather. Node-wise MLP per 128-node block: PE transpose to
    feature-major, two fp16 matmuls with tanh (ACT bias), written
    feature-major; host restores order.
"""

import numpy as np

P = 128
NCORES = 8
DIN = 48
DHID = 128
NPAD = 100352
NLOC = NPAD // NCORES      # 12544
NB = NLOC // P             # 98
CHUNK = NPAD // 4          # 25088
NCLS = 8
GMAX = 1024                # dma_gather / dma_scatter_add index limit
GCOLS = GMAX // P          # 8 columns per gather instruction
TILE_COLS = 32             # bulk G-tile columns (4 gather instrs)
NRANGE = 4                 # mi accumulator ranges for MLP pipelining
QUANT_LAMBDA = 2           # DP grouping: column-cost of an extra fold group
COST_SLOT = 1.45            # rel cost of one bulk slot (Pool+DMA+DVE)
COST_OVF = 4.5             # rel cost of one overflow edge

_PROG_CACHE: dict = {}


# --------------------------------------------------------------------------
# host prep
# --------------------------------------------------------------------------

def _class_assign(dst, src):
    """2-choice greedy per-destination class balancing. Returns cls [E]."""
    E = len(dst)
    cA = src // CHUNK
    cB = 4 + (src % 4)
    sortp = np.argsort(dst, kind="stable")
    degd = np.bincount(dst, minlength=NPAD)
    first = np.cumsum(degd) - degd
    srank_d = np.arange(E) - first[dst[sortp]]
    loads = np.zeros((NPAD, NCLS), np.int32)
    cls = np.empty(E, np.int64)
    maxr = int(srank_d.max()) if E else 0
    for r in range(maxr + 1):
        sel = sortp[srank_d == r]
        if len(sel) == 0:
            continue
        d = dst[sel]
        la = loads[d, cA[sel]]
        lb = loads[d, cB[sel]]
        pick_b = lb < la
        c = np.where(pick_b, cB[sel], cA[sel])
        loads[d, c] += 1
        cls[sel] = c
    return cls


def _quantize_groups(T_col, cuts):
    """DP: partition blocks into contiguous groups (respecting forced cuts),
    pad T to group max. Returns [(b0, nbs, Tq)] with Tq>0 only."""
    groups = []
    for ci in range(len(cuts) - 1):
        lo, hi = cuts[ci], cuts[ci + 1]
        n = hi - lo
        if n <= 0:
            continue
        seg = T_col[lo:hi]
        INF = 1 << 60
        best = [INF] * (n + 1)
        prev = [0] * (n + 1)
        best[0] = 0
        for j in range(1, n + 1):
            m = 0
            ssum = 0
            for i in range(j - 1, -1, -1):
                if seg[i] > m:
                    m = seg[i]
                ssum += seg[i]
                cost = best[i] + (m * (j - i) - ssum) + QUANT_LAMBDA
                if cost < best[j]:
                    best[j] = cost
                    prev[j] = i
        bounds = []
        j = n
        while j > 0:
            bounds.append((prev[j], j))
            j = prev[j]
        for i, j in reversed(bounds):
            Tq = int(seg[i:j].max())
            if Tq > 0:
                groups.append((lo + i, j - i, Tq))
    return groups


def _wrap_idx(vals16, pos):
    """Scatter int16 values into the wrapped-by-16, 8x-replicated layout.
    vals16 [n], pos [n] list positions. Returns writes for a [16, L] array:
    (rows, cols)."""
    return (pos % 16).astype(np.int64), pos // 16


def _build_direction(dst, src, w, deg, core, ranges):
    """Full per-direction layout: bulk bands + overflow lists."""
    loc = np.empty(NPAD, np.int64)
    nodes_by_core = []
    for k in range(NCORES):
        nodes_k = np.where(core == k)[0]
        lk = nodes_k[np.argsort(-deg[nodes_k], kind="stable")]
        loc[lk] = np.arange(NLOC)
        nodes_by_core.append(lk)
    blk = loc // P
    part = loc % P

    cls = _class_assign(dst, src)
    E = len(dst)

    # per-edge rank within (dest, class)
    key = dst * NCLS + cls
    sortp = np.argsort(key, kind="stable")
    ks = key[sortp]
    cnt = np.bincount(ks, minlength=NPAD * NCLS)
    firstk = np.cumsum(cnt) - cnt
    srank = np.empty(E, np.int64)
    srank[sortp] = np.arange(E) - firstk[ks]

    loads = cnt.reshape(NPAD, NCLS)

    # T* per (block, class): minimize 128*T*COST_SLOT + unified_ovf(T)*COST_OVF
    maxl = int(loads.max())
    # histogram of per-(core, block, class) dest-loads
    hist = np.zeros((NCORES, NB, NCLS, maxl + 1), np.int64)
    nodes = np.arange(NPAD)
    for c in range(NCLS):
        np.add.at(hist, (core[nodes], blk[nodes], c, loads[nodes, c]), 1)
    # abovec[..., lv] = #dests with load > lv ; ovf(T) = sum_{lv>=T} abovec[lv]
    cum = hist.cumsum(axis=3)
    total = cum[..., -1:]
    abovec = (total - cum)  # [..., lv] = #dests with load > lv
    suf = abovec[..., ::-1].cumsum(axis=3)[..., ::-1]  # per-core ovf at T=lv
    ovf_unified = suf.max(axis=0)  # [NB, NCLS, maxl+1]
    Tcost = (128 * np.arange(maxl + 1)[None, None, :] * COST_SLOT
             + ovf_unified * COST_OVF)
    Tstar = Tcost.argmin(axis=2)  # [NB, NCLS]

    cuts = sorted(set([0, NB] + list(ranges)))
    groups_per_cls = [_quantize_groups(Tstar[:, c], cuts) for c in range(NCLS)]
    Tq = np.zeros((NB, NCLS), np.int64)
    for c in range(NCLS):
        for b0, nbs, t in groups_per_cls[c]:
            Tq[b0 : b0 + nbs, c] = t

    # ---- bulk tile packing ----
    # Per class: one continuous column run over all its groups (block order),
    # split into tiles of TILE_COLS; gather windows stay full except at each
    # class's tail. Tiles are interleaved across classes by block progress so
    # mi ranges complete roughly in order.
    cls_tiles = {c: [] for c in range(NCLS)}
    colstart = np.full((NB, NCLS), -1, np.int64)
    col = 0
    for c in range(NCLS):
        runs = []
        for b0, nbs, t in groups_per_cls[c]:
            nmax = max(1, TILE_COLS // t)
            j = 0
            while j < nbs:
                nn = min(nmax, nbs - j)
                runs.append((b0 + j, nn, t))
                j += nn
        cur = None
        for b0, nbs, t in runs:
            need = nbs * t
            if cur is None or cur["cols"] + need > TILE_COLS:
                if cur is not None:
                    cls_tiles[c].append(cur)
                cur = dict(cls=c, col0=col, cols=0, entries=[])
            cur["entries"].append((t, b0, nbs, cur["cols"]))
            for i in range(nbs):
                colstart[b0 + i, c] = col + i * t
            cur["cols"] += need
            col += need
        if cur is not None:
            cls_tiles[c].append(cur)
    CT = col
    # interleave tiles by first-block progress
    tiles = sorted(
        (t for c in range(NCLS) for t in cls_tiles[c]),
        key=lambda t: (t["entries"][0][1], t["cls"]),
    )

    percol_col0 = np.zeros(max(CT, 1), np.int64)
    for t in tiles:
        percol_col0[t["col0"] : t["col0"] + t["cols"]] = t["col0"]

    is_bulk = srank < Tq[blk[dst], cls]

    # class-local index values
    val = np.where(cls < 4, src - (cls * CHUNK), src // 4).astype(np.int16)

    eb = np.where(is_bulk)[0]
    d = dst[eb]
    k_e = core[d]
    b_e = blk[d]
    p_e = part[d]
    colg = colstart[b_e, cls[eb]] + srank[eb]
    col0 = percol_col0[colg]
    g_t = (colg - col0) * P + p_e
    ic = col0 * 8 + g_t // 16
    ir = g_t % 16

    idx16 = np.zeros((NCORES, 16, max(CT, 1) * 8), np.int16)
    idx16[k_e, ir, ic] = val[eb]
    idx_arr = np.tile(idx16, (1, 8, 1))
    w_arr = np.zeros((NCORES, P, max(CT, 1)), np.float16)
    w_arr[k_e, p_e, colg] = w[eb].astype(np.float16)

    # ---- overflow lists ----
    # order: (rank', class, dest); windows of GMAX cut at rank boundaries
    eo = np.where(~is_bulk)[0]
    # global per-dest overflow rank (across classes): scatter windows slice by
    # rank, so a dest must appear at most once per rank block
    osort0 = np.argsort(dst[eo], kind="stable")
    dso = dst[eo][osort0]
    dcnt = np.bincount(dso, minlength=NPAD)
    dfirst = np.cumsum(dcnt) - dcnt
    rank2 = np.empty(len(eo), np.int64)
    rank2[osort0] = np.arange(len(eo)) - dfirst[dso]
    # rotate each dest's rank slots (stays a permutation per dest) so the
    # per-(rank, class) segment sizes balance across cores
    kd = dcnt[dst[eo]]
    rank2 = np.where(kd > 0, (rank2 + dst[eo]) % np.maximum(kd, 1), rank2)
    # per-core list; unified layout = same windows/segments across cores, so
    # build per-core orderings with shared per-(rank, class) segment sizes
    segcnt = np.zeros((NCORES, int(rank2.max()) + 1 if len(eo) else 1, NCLS),
                      np.int64)
    if len(eo):
        np.add.at(segcnt, (core[dst[eo]], rank2, cls[eo]), 1)
    useg = segcnt.max(axis=0)  # unified per-(rank, class) segment length
    # pad each segment to a 128 multiple: every gather instruction restarts
    # its list at partition 0, so segments must start 128-aligned
    useg = -(-useg // P) * P
    NRK = useg.shape[0]
    seg_off = np.zeros((NRK, NCLS), np.int64)
    posn = 0
    ov_windows = []   # pos0 per GMAX window
    ov_gathers = []   # (cls, pos0, n) per gather instruction (<= GMAX, class-pure)
    for r in range(NRK):
        for c in range(NCLS):
            n = int(useg[r, c])
            if n == 0:
                continue
            seg_off[r, c] = posn
            s = 0
            while s < n:
                woff = (posn + s) % GMAX
                take = min(n - s, GMAX - woff)
                ov_gathers.append((c, posn + s, take))
                s += take
            posn += n
        posn = -(-posn // GMAX) * GMAX  # pad rank block to window multiple
    OVT = posn // P  # overflow columns
    for wpos in range(0, posn, GMAX):
        ov_windows.append(wpos)

    ov_idx16 = np.zeros((NCORES, 16, max(OVT, 1) * 8), np.int16)
    ov_sidx16 = np.full((NCORES, 16, max(OVT, 1) * 8), NLOC, np.int16)
    ov_w = np.zeros((NCORES, P, max(OVT, 1)), np.float16)
    if len(eo):
        # per-core position within segment: stable order by (core,rank,cls)
        okey = (core[dst[eo]] * NRK + rank2) * NCLS + cls[eo]
        osort = np.argsort(okey, kind="stable")
        oks = okey[osort]
        ocnt = np.bincount(oks, minlength=NCORES * NRK * NCLS)
        ofirst = np.cumsum(ocnt) - ocnt
        opos = np.empty(len(eo), np.int64)
        opos[osort] = np.arange(len(eo)) - ofirst[oks]
        e = eo
        posg = seg_off[rank2, cls[e]] + opos  # list position
        kk = core[dst[e]]
        ov_idx16[kk, posg % 16, (posg // 16)] = val[e]
        ov_sidx16[kk, posg % 16, (posg // 16)] = loc[dst[e]].astype(np.int16)
        ov_w[kk, posg % P, posg // P] = w[e].astype(np.float16)
    ov_idx = np.tile(ov_idx16, (1, 8, 1))
    ov_sidx = np.tile(ov_sidx16, (1, 8, 1))

    return dict(
        loc=loc, blk=blk, part=part, nodes_by_core=nodes_by_core,
        tiles=tiles, CT=CT, idx_arr=idx_arr, w_arr=w_arr, Tq=Tq,
        OVT=OVT, ov_idx=ov_idx, ov_sidx=ov_sidx, ov_w=ov_w,
        ov_windows=ov_windows, ov_gathers=ov_gathers,
    )


def _host_prep(x, edge_index, edge_attr):
    N = x.shape[0]
    row = np.asarray(edge_index[0]).astype(np.int64)
    col = np.asarray(edge_index[1]).astype(np.int64)
    w = np.asarray(edge_attr, dtype=np.float32).reshape(-1)

    deg_in = np.bincount(col, minlength=NPAD)
    deg_out = np.bincount(row, minlength=NPAD)
    order = np.argsort(-(deg_in + deg_out), kind="stable")
    rank = np.empty(NPAD, np.int64)
    rank[order] = np.arange(NPAD)
    core = (rank % NCORES).astype(np.int64)

    ranges = [30, 60, 90]

    dmi = _build_direction(col, row, w, deg_in, core, ranges)
    dmo = _build_direction(row, col, w, deg_out, core, [])

    xf = np.zeros((NPAD, DIN), np.float32)
    xf[:N] = np.asarray(x, np.float32)
    x16 = xf.astype(np.float16)
    tabs = np.zeros((NCLS, CHUNK, 128), np.float16)
    for c in range(4):
        tabs[c, :, :DIN] = x16[c * CHUNK : (c + 1) * CHUNK]
    for r in range(4):
        tabs[4 + r, :, :DIN] = x16[r::4]

    x_own = np.zeros((NCORES, P, NB * DIN), np.float16)
    for k in range(NCORES):
        lk = dmi["nodes_by_core"][k]
        xv = x16[lk]
        x_own[k] = (
            xv.reshape(NB, P, DIN).transpose(1, 0, 2).reshape(P, NB * DIN)
        )

    realign = np.zeros((NCORES, 16, NLOC // 16), np.int16)
    g = np.arange(NLOC)
    for k in range(NCORES):
        lk = dmi["nodes_by_core"][k]
        vals = dmo["loc"][lk].astype(np.int16)
        realign[k, g % 16, g // 16] = vals
    realign = np.tile(realign, (1, 8, 1))

    return dict(
        N=N, core=core, dmi=dmi, dmo=dmo, tabs=tabs, x_own=x_own,
        realign=realign,
    )


# --------------------------------------------------------------------------
# numpy emulation (layout validation)
# --------------------------------------------------------------------------

def _emulate_agg(meta, direction):
    """Emulate both tiers -> acc [NCORES, 128, NB*48] fp32 in local order."""
    d = meta[direction]
    tabs = meta["tabs"]
    CT = d["CT"]
    acc = np.zeros((NCORES, P, NB, DIN), np.float32)
    colcls = np.zeros(max(CT, 1), np.int64)
    colblk = np.zeros(max(CT, 1), np.int64)
    col0a = np.zeros(max(CT, 1), np.int64)
    for t in d["tiles"]:
        col0a[t["col0"] : t["col0"] + t["cols"]] = t["col0"]
        for s, b0, nbs, cbase in t["entries"]:
            for i in range(nbs):
                c0 = t["col0"] + cbase + i * s
                colcls[c0 : c0 + s] = t["cls"]
                colblk[c0 : c0 + s] = b0 + i
    for k in range(NCORES):
        if CT:
            idx = d["idx_arr"][k]
            wv = d["w_arr"][k]
            cols = np.arange(CT)
            pp = np.arange(P)
            g_t = (cols[None, :] - col0a[None, :]) * P + pp[:, None]
            ic = col0a[None, :] * 8 + g_t // 16
            ir = g_t % 16
            vals = idx[ir, ic].astype(np.int64)
            gathered = tabs[colcls[None, :].repeat(P, 0), vals][:, :, :DIN]
            prod = gathered.astype(np.float16) * wv[:, :, None]
            np.add.at(acc[k], (slice(None), colblk), prod.astype(np.float32))
        # overflow
        OVT = d["OVT"]
        if OVT:
            oi = d["ov_idx"][k]
            os_ = d["ov_sidx"][k]
            ow = d["ov_w"][k]
            # reconstruct per-position
            ocls = np.zeros(OVT * P, np.int64)
            for c, pos0, n in d["ov_gathers"]:
                ocls[pos0 : pos0 + n] = c
            g = np.arange(OVT * P)
            vals = oi[g % 16, g // 16].astype(np.int64)
            sidx = os_[g % 16, g // 16].astype(np.int64)
            wvals = ow[g % P, g // P].astype(np.float16)
            gath = tabs[ocls, vals][:, :DIN].astype(np.float16)
            prod = (gath * wvals[:, None]).astype(np.float32)
            sel = sidx < NLOC
            tgt = sidx[sel]
            np.add.at(acc[k], (tgt % P, tgt // P), prod[sel])
    return acc.reshape(NCORES, P, NB * DIN)


def _emulate(meta, W1, b1, W2, b2):
    mi = _emulate_agg(meta, "dmi")
    mo = _emulate_agg(meta, "dmo")
    out = np.zeros((NPAD, DHID), np.float32)
    for k in range(NCORES):
        lk_i = meta["dmi"]["nodes_by_core"][k]
        lk_o = meta["dmo"]["nodes_by_core"][k]
        mi_k = mi[k].reshape(P, NB, DIN).transpose(1, 0, 2).reshape(NLOC, DIN)
        mo_k = mo[k].reshape(P, NB, DIN).transpose(1, 0, 2).reshape(NLOC, DIN)
        mo_full = np.zeros((NLOC, DIN), np.float32)
        mo_full[:] = mo_k  # mo-local order
        mo2_k = mo_full[meta["dmo"]["loc"][lk_i]]
        x_k = meta["x_own"][k].reshape(P, NB, DIN).transpose(1, 0, 2).reshape(
            NLOC, DIN
        ).astype(np.float32)
        M = np.concatenate([mi_k, mo2_k, x_k], axis=1)
        h = np.tanh(M @ W1.T + b1)
        out[lk_i] = np.tanh(h @ W2.T + b2)
    return out[: meta["N"]]


# --------------------------------------------------------------------------
# device program
# --------------------------------------------------------------------------

def _dma_gather96(gp, mybir, out_ap, in_ap, idxs_ap, num_idxs):
    from concourse.bass import exact_div

    elem_step = in_ap.ap[0][0]
    stride_bytes = elem_step * mybir.dt.size(in_ap.dtype)
    return gp.add_instruction(
        mybir.InstDMAGatherAnt(
            name=gp.bass.get_next_instruction_name(),
            ins=[
                *gp.lower_ap_dma(in_ap, for_custom_bir_dma=True),
                gp.lower_ap(idxs_ap),
                gp.lower_val_access(gp.to_reg(num_idxs)),
            ],
            outs=[gp.lower_ap(out_ap)],
            transpose=False,
            num_idxs=num_idxs,
            elem_size=DIN,
            stride_bytes_256=exact_div(stride_bytes, 256),
            gen_mode=0,
            single_packet=True,
            queue_num=0,
            sbuf_tokens_per_rank=0,
            sbuf_free_dim_per_rank=0,
            sbuf_free_dim_pad_per_rank=0,
            sbuf_byte_offset=0,
        )
    )


def _build_program(meta):
    import concourse.bacc as bacc
    import concourse.bass as bass
    import concourse.mybir as mybir
    import concourse.tile as tile
    from concourse.masks import make_identity

    f32 = mybir.dt.float32
    f16 = mybir.dt.float16
    i16 = mybir.dt.int16

    dmi, dmo = meta["dmi"], meta["dmo"]
    CTI, CTO = max(dmi["CT"], 1), max(dmo["CT"], 1)
    OVI, OVO = max(dmi["OVT"], 1), max(dmo["OVT"], 1)

    nc = bacc.Bacc(
        "TRN2",
        target_bir_lowering=False,
        debug=False,
        num_devices=NCORES,
        dynamic_dma_scratch_size=65536,
    )

    tabs_d = [
        nc.dram_tensor(f"tab{c}", [CHUNK, 128], f16, kind="ExternalInput")
        for c in range(NCLS)
    ]
    idx_mi_d = nc.dram_tensor("idx_mi", [P, CTI * 8], i16, kind="ExternalInput")
    idx_mo_d = nc.dram_tensor("idx_mo", [P, CTO * 8], i16, kind="ExternalInput")
    w_mi_d = nc.dram_tensor("w_mi", [P, CTI], f16, kind="ExternalInput")
    w_mo_d = nc.dram_tensor("w_mo", [P, CTO], f16, kind="ExternalInput")
    ovi_idx_d = nc.dram_tensor("ovi_idx", [P, OVI * 8], i16, kind="ExternalInput")
    ovi_sidx_d = nc.dram_tensor("ovi_sidx", [P, OVI * 8], i16, kind="ExternalInput")
    ovi_w_d = nc.dram_tensor("ovi_w", [P, OVI], f16, kind="ExternalInput")
    ovo_idx_d = nc.dram_tensor("ovo_idx", [P, OVO * 8], i16, kind="ExternalInput")
    ovo_sidx_d = nc.dram_tensor("ovo_sidx", [P, OVO * 8], i16, kind="ExternalInput")
    ovo_w_d = nc.dram_tensor("ovo_w", [P, OVO], f16, kind="ExternalInput")
    x_own_d = nc.dram_tensor("x_own", [P, NB * DIN], f16, kind="ExternalInput")
    realign_d = nc.dram_tensor("realign", [P, NLOC // 16], i16, kind="ExternalInput")
    mi_acc = nc.dram_tensor("mi_acc", [NLOC + P, 128], f16, kind="Internal")
    mo_acc = nc.dram_tensor("mo_acc", [NLOC + P, 128], f16, kind="Internal")
    w1ta_d = nc.dram_tensor("w1ta", [DIN, DHID], f16, kind="ExternalInput")
    w1tb_d = nc.dram_tensor("w1tb", [DIN, DHID], f16, kind="ExternalInput")
    w1tc_d = nc.dram_tensor("w1tc", [DIN, DHID], f16, kind="ExternalInput")
    w2t_d = nc.dram_tensor("w2t", [DHID, DHID], f16, kind="ExternalInput")
    b1_d = nc.dram_tensor("b1", [DHID, 1], f32, kind="ExternalInput")
    b2_d = nc.dram_tensor("b2", [DHID, 1], f32, kind="ExternalInput")
    out_t = nc.dram_tensor("out_t", [P, NLOC], f16, kind="ExternalOutput")

    rbounds = [0, 30, 60, 90, NB]

    with tile.TileContext(nc) as tc:
        with (
            tc.tile_pool(name="const", bufs=1) as const,
            tc.tile_pool(name="gidx", bufs=4) as gidx,
            tc.tile_pool(name="gpool", bufs=4) as gpool,
            tc.tile_pool(name="mlp", bufs=4) as mlp,
            tc.tile_pool(name="ost", bufs=2) as ostp,
            tc.tile_pool(name="psT", bufs=3, space="PSUM") as psT,
            tc.tile_pool(name="psH", bufs=2, space="PSUM") as psH,
        ):
            w_mi_sb = const.tile([P, CTI], f16)
            nc.sync.dma_start(w_mi_sb[:], w_mi_d[:])
            w_mo_sb = const.tile([P, CTO], f16)
            nc.sync.dma_start(w_mo_sb[:], w_mo_d[:])
            ovi_w_sb = const.tile([P, OVI], f16)
            nc.sync.dma_start(ovi_w_sb[:], ovi_w_d[:])
            ovo_w_sb = const.tile([P, OVO], f16)
            nc.sync.dma_start(ovo_w_sb[:], ovo_w_d[:])
            realign_sb = const.tile([P, NLOC // 16], i16)
            nc.sync.dma_start(realign_sb[:], realign_d[:])
            w1ta_sb = const.tile([DIN, DHID], f16)
            nc.sync.dma_start(w1ta_sb[:], w1ta_d[:])
            w1tb_sb = const.tile([DIN, DHID], f16)
            nc.sync.dma_start(w1tb_sb[:], w1tb_d[:])
            w1tc_sb = const.tile([DIN, DHID], f16)
            nc.sync.dma_start(w1tc_sb[:], w1tc_d[:])
            w2t_sb = const.tile([DHID, DHID], f16)
            nc.sync.dma_start(w2t_sb[:], w2t_d[:])
            b1_sb = const.tile([DHID, 1], f32)
            nc.sync.dma_start(b1_sb[:], b1_d[:])
            b2_sb = const.tile([DHID, 1], f32)
            nc.sync.dma_start(b2_sb[:], b2_d[:])
            ident = const.tile([P, P], f16)
            make_identity(nc, ident[:])
            xo_sb = const.tile([P, NB * DIN], f16)
            nc.sync.dma_start(xo_sb[:], x_own_d[:])

            mo_sb = const.tile([P, NB * DIN], f16)
            mo2_sb = const.tile([P, NB * DIN], f16)
            mi_rs = []
            for ri in range(NRANGE):
                mi_ri = const.tile([P, (rbounds[ri + 1] - rbounds[ri]) * DIN],
                                   f16, name=f"mi_r{ri}")
                mi_rs.append(mi_ri)
                nc.vector.memset(mi_ri[:], 0.0)
            zz = const.tile([P, (NB + 1) * DIN], f16)
            nc.vector.memset(mo_sb[:], 0.0)
            nc.vector.memset(zz[:], 0.0)
            # zero DRAM accumulators (NLOC + dummy block rows)
            for acc_d in (mi_acc, mo_acc):
                nc.sync.dma_start(
                    acc_d[:, 0:DIN].rearrange("(b p) f -> p b f", p=P),
                    zz[:].rearrange("p (b f) -> p b f", f=DIN),
                )

            def emit_interleaved(dirmeta, idx_d, w_sb, acc_of,
                                 oidx_d, osidx_d, ow_sb, acc_d,
                                 only_range=None):
                tiles = [t for t in dirmeta["tiles"]
                         if only_range is None
                         or only_range[0] <= t["entries"][0][1] < only_range[1]]
                wins = list(dirmeta["ov_windows"]) if only_range is None or \
                    only_range[0] == 0 else []
                step = max(1, (len(tiles) + len(wins)) // max(len(wins), 1)) \
                    if wins else 1 << 30
                wi = 0
                for i, t in enumerate(tiles):
                    emit_one_bulk(dirmeta, t, idx_d, w_sb, acc_of)
                    if wi < len(wins) and i % step == step - 1:
                        emit_overflow(dirmeta, oidx_d, osidx_d, ow_sb, acc_d,
                                      [wins[wi]])
                        wi += 1
                if wi < len(wins):
                    emit_overflow(dirmeta, oidx_d, osidx_d, ow_sb, acc_d,
                                  wins[wi:])

            def emit_bulk(dirmeta, idx_d, w_sb, acc_of, only_range=None):
                rlo, rhi = (0, NB) if only_range is None else only_range
                for t in dirmeta["tiles"]:
                    b_first = t["entries"][0][1]
                    if not (rlo <= b_first < rhi):
                        continue
                    emit_one_bulk(dirmeta, t, idx_d, w_sb, acc_of)

            def emit_one_bulk(dirmeta, t, idx_d, w_sb, acc_of):
                    cols = t["cols"]
                    c = t["cls"]
                    col0 = t["col0"]
                    idx_sb = gidx.tile([P, TILE_COLS * 8], i16, tag="gi")
                    nc.sync.dma_start(
                        idx_sb[:, : cols * 8],
                        idx_d[:, col0 * 8 : (col0 + cols) * 8],
                    )
                    G = gpool.tile([P, TILE_COLS * DIN], f16, tag="G")
                    for q0 in range(0, cols, GCOLS):
                        qn = min(GCOLS, cols - q0)
                        _dma_gather96(
                            nc.gpsimd, mybir,
                            out_ap=G[:, q0 * DIN : (q0 + qn) * DIN].rearrange(
                                "p (c f) -> p c f", f=DIN
                            ),
                            in_ap=tabs_d[c][:, 0:DIN],
                            idxs_ap=idx_sb[:, q0 * 8 : (q0 + qn) * 8],
                            num_idxs=qn * P,
                        )
                    g3 = G[:, : cols * DIN].rearrange("p (c f) -> p c f", f=DIN)
                    wv = w_sb[:, col0 : col0 + cols]
                    wb = bass.AP(
                        wv.tensor,
                        wv.offset,
                        [list(wv.ap[0]), list(wv.ap[1]), [0, DIN]],
                    )
                    nc.vector.tensor_tensor(
                        out=g3, in0=g3, in1=wb, op=mybir.AluOpType.mult
                    )
                    for s, b0, nbs, cbase in t["entries"]:
                        gg = G[
                            :, cbase * DIN : (cbase + nbs * s) * DIN
                        ].rearrange("p (b s f) -> p b s f", s=s, f=DIN)
                        ss = s
                        while ss > 1:
                            half = ss // 2
                            hi0 = ss - half
                            nc.vector.tensor_tensor(
                                out=gg[:, :, 0:half, :],
                                in0=gg[:, :, 0:half, :],
                                in1=gg[:, :, hi0:ss, :],
                                op=mybir.AluOpType.add,
                            )
                            ss = hi0
                        acc_sb, boff = acc_of(b0)
                        accv = acc_sb[
                            :, (b0 - boff) * DIN : (b0 - boff + nbs) * DIN
                        ].rearrange("p (b f) -> p b f", f=DIN)
                        nc.vector.tensor_tensor(
                            out=accv, in0=accv, in1=gg[:, :, 0, :],
                            op=mybir.AluOpType.add,
                        )

            def emit_overflow(dirmeta, oidx_d, osidx_d, ow_sb, acc_d,
                              subset=None):
                gathers = dirmeta["ov_gathers"]
                windows = dirmeta["ov_windows"] if subset is None else subset
                for wpos in windows:
                    gi = 0
                    while gi < len(gathers) and gathers[gi][1] < wpos:
                        gi += 1
                    idx_sb = gidx.tile([P, GCOLS * 8], i16, tag="oi")
                    nc.sync.dma_start(
                        idx_sb[:],
                        oidx_d[:, (wpos // 16) : (wpos // 16) + GCOLS * 8],
                    )
                    sidx_sb = gidx.tile([P, GCOLS * 8], i16, tag="os")
                    nc.sync.dma_start(
                        sidx_sb[:],
                        osidx_d[:, (wpos // 16) : (wpos // 16) + GCOLS * 8],
                    )
                    G = gpool.tile([P, GCOLS * DIN], f16, tag="G")
                    while gi < len(gathers) and gathers[gi][1] < wpos + GMAX:
                        c, pos0, n = gathers[gi]
                        lo = pos0 - wpos
                        _dma_gather96(
                            nc.gpsimd, mybir,
                            out_ap=G[
                                :, (lo // P) * DIN : ((lo + n) // P) * DIN
                            ].rearrange("p (c f) -> p c f", f=DIN),
                            in_ap=tabs_d[c][:, 0:DIN],
                            idxs_ap=idx_sb[:, lo // 16 : (lo + n) // 16],
                            num_idxs=n,
                        )
                        gi += 1
                    g3 = G[:].rearrange("p (c f) -> p c f", f=DIN)
                    wv = ow_sb[:, wpos // P : wpos // P + GCOLS]
                    wb = bass.AP(
                        wv.tensor,
                        wv.offset,
                        [list(wv.ap[0]), list(wv.ap[1]), [0, DIN]],
                    )
                    nc.vector.tensor_tensor(
                        out=g3, in0=g3, in1=wb, op=mybir.AluOpType.mult
                    )
                    nc.gpsimd.dma_scatter_add(
                        out_ap=acc_d[:, 0:DIN],
                        in_ap=g3,
                        idxs_ap=sidx_sb[:],
                        num_idxs=GMAX,
                        num_idxs_reg=GMAX,
                        elem_size=DIN,
                        elem_step=128,
                    )

            def acc_mo(b0):
                return mo_sb, 0

            def acc_mi(b0):
                for ri in range(NRANGE):
                    if b0 < rbounds[ri + 1]:
                        return mi_rs[ri], rbounds[ri]
                raise AssertionError(b0)

            # ---- mo: bulk with overflow windows interleaved ----
            emit_interleaved(dmo, idx_mo_d, w_mo_sb, acc_mo,
                             ovo_idx_d, ovo_sidx_d, ovo_w_sb, mo_acc)
            # merge: mo_sb += mo_acc; write back
            tmp = const.tile([P, NB * DIN], f16)
            nc.sync.dma_start(
                tmp[:].rearrange("p (b f) -> p b f", f=DIN),
                mo_acc[0:NLOC, 0:DIN].rearrange("(b p) f -> p b f", p=P),
            )
            nc.vector.tensor_tensor(
                out=mo_sb[:], in0=mo_sb[:], in1=tmp[:], op=mybir.AluOpType.add
            )
            nc.sync.dma_start(
                mo_acc[0:NLOC, 0:DIN].rearrange("(b p) f -> p b f", p=P),
                mo_sb[:].rearrange("p (b f) -> p b f", f=DIN),
            )

            def emit_mlp(rlo, rhi):
                OG = 4
                for b0 in range(rlo, rhi, OG):
                    og = min(OG, rhi - b0)
                    os_ = ostp.tile([P, OG * P], f16, tag="oo")
                    for j in range(og):
                        b = b0 + j
                        hp = psH.tile([P, P], f32, tag="hp")
                        mi_t, mi_b0 = acc_mi(b)
                        for q, (src_sb, bb, w1q) in enumerate((
                            (mi_t, b - mi_b0, w1ta_sb),
                            (mo2_sb, b, w1tb_sb),
                            (xo_sb, b, w1tc_sb),
                        )):
                            pA = psT.tile([DIN, P], f16, tag="pA")
                            nc.tensor.transpose(
                                pA[:], src_sb[:, bb * DIN : (bb + 1) * DIN],
                                ident[:],
                            )
                            mt = mlp.tile([DIN, P], f16, tag="mt")
                            nc.vector.tensor_copy(out=mt[:], in_=pA[:])
                            nc.tensor.matmul(
                                hp[:], w1q[:], mt[:],
                                start=(q == 0), stop=(q == 2),
                            )
                        hs = mlp.tile([P, P], f16, tag="hs")
                        nc.scalar.activation(
                            hs[:], hp[:],
                            mybir.ActivationFunctionType.Tanh,
                            bias=b1_sb[:], scale=1.0,
                        )
                        op_ = psH.tile([P, P], f32, tag="op")
                        nc.tensor.matmul(
                            op_[:], w2t_sb[:], hs[:], start=True, stop=True
                        )
                        nc.scalar.activation(
                            os_[:, j * P : (j + 1) * P], op_[:],
                            mybir.ActivationFunctionType.Tanh,
                            bias=b2_sb[:], scale=1.0,
                        )
                    nc.sync.dma_start(
                        out_t[:, b0 * P : (b0 + og) * P], os_[:, : og * P]
                    )

            realigned = False
            for ri in range(NRANGE):
                rlo, rhi = rbounds[ri], rbounds[ri + 1]
                if ri == 0:
                    emit_interleaved(dmi, idx_mi_d, w_mi_sb, acc_mi,
                                     ovi_idx_d, ovi_sidx_d, ovi_w_sb, mi_acc,
                                     (rlo, rhi))
                else:
                    emit_bulk(dmi, idx_mi_d, w_mi_sb, acc_mi, (rlo, rhi))
                if not realigned:
                    # realign mo to mi-local order (after some mi bulk so the
                    # Pool queue is not head-of-line blocked on the mo merge)
                    for g0 in range(0, NLOC, GMAX):
                        gn = min(GMAX, NLOC - g0)
                        _dma_gather96(
                            nc.gpsimd, mybir,
                            out_ap=mo2_sb[
                                :, (g0 // P) * DIN : ((g0 + gn) // P) * DIN
                            ].rearrange("p (b f) -> p b f", f=DIN),
                            in_ap=mo_acc[:, 0:DIN],
                            idxs_ap=realign_sb[:, g0 // 16 : (g0 + gn) // 16],
                            num_idxs=gn,
                        )
                    realigned = True
                # merge overflow acc for this range
                tmpr_full = gpool.tile([P, 30 * DIN], f16, tag="mr")
                tmpr = tmpr_full[:, : (rhi - rlo) * DIN]
                nc.sync.dma_start(
                    tmpr[:].rearrange("p (b f) -> p b f", f=DIN),
                    mi_acc[rlo * P : rhi * P, 0:DIN].rearrange(
                        "(b p) f -> p b f", p=P
                    ),
                )
                nc.vector.tensor_tensor(
                    out=mi_rs[ri][:],
                    in0=mi_rs[ri][:],
                    in1=tmpr[:],
                    op=mybir.AluOpType.add,
                )
                emit_mlp(rlo, rhi)

    nc.compile()
    return nc


# --------------------------------------------------------------------------
# entry point
# --------------------------------------------------------------------------

def kernel(x, edge_index, edge_attr, W1, b1, W2, b2):
    x = np.asarray(x, np.float32)
    meta = _host_prep(x, edge_index, edge_attr)
    dmi, dmo = meta["dmi"], meta["dmo"]
    key = (meta["N"], dmi["CT"], dmo["CT"], dmi["OVT"], dmo["OVT"],
           tuple(t["col0"] for t in dmi["tiles"]),
           tuple(t["col0"] for t in dmo["tiles"]),
           tuple(dmi["ov_gathers"]), tuple(dmo["ov_gathers"]))
    if key not in _PROG_CACHE:
        _PROG_CACHE[key] = _build_program(meta)
    nc = _PROG_CACHE[key]

    W1 = np.asarray(W1, np.float32)
    W2 = np.asarray(W2, np.float32)
    b1v = np.asarray(b1, np.float32).reshape(DHID, 1)
    b2v = np.asarray(b2, np.float32).reshape(DHID, 1)
    w1t = np.ascontiguousarray(W1.T)
    w1ta = np.ascontiguousarray(w1t[:DIN]).astype(np.float16)
    w1tb = np.ascontiguousarray(w1t[DIN : 2 * DIN]).astype(np.float16)
    w1tc = np.ascontiguousarray(w1t[2 * DIN :]).astype(np.float16)
    w2t = np.ascontiguousarray(W2.T).astype(np.float16)

    in_maps = []
    for k in range(NCORES):
        m = {
            "idx_mi": dmi["idx_arr"][k],
            "idx_mo": dmo["idx_arr"][k],
            "w_mi": dmi["w_arr"][k],
            "w_mo": dmo["w_arr"][k],
            "ovi_idx": dmi["ov_idx"][k],
            "ovi_sidx": dmi["ov_sidx"][k],
            "ovi_w": dmi["ov_w"][k],
            "ovo_idx": dmo["ov_idx"][k],
            "ovo_sidx": dmo["ov_sidx"][k],
            "ovo_w": dmo["ov_w"][k],
            "x_own": meta["x_own"][k],
            "realign": meta["realign"][k],
            "w1ta": w1ta, "w1tb": w1tb, "w1tc": w1tc, "w2t": w2t,
            "b1": b1v, "b2": b2v,
        }
        for c in range(NCLS):
            m[f"tab{c}"] = meta["tabs"][c]
        in_maps.append(m)

    runner = _get_runner(nc)
    results = runner.run(in_maps)
    global _LAST
    _LAST = (nc, in_maps)

    out = np.empty((NPAD, DHID), np.float32)
    for k in range(NCORES):
        out[dmi["nodes_by_core"][k]] = results[k]["out_t"].T.astype(np.float32)
    return out[: meta["N"]]


_LAST = None
_RUNNER_CACHE: dict = {}


class _PjrtRunner:
    """Builds the shard_map-jitted NEFF executor once; supports repeated
    dispatches with device-resident inputs for timing."""

    def __init__(self, nc):
        import jax
        import jax.numpy as jnp
        import concourse.mybir as mybir
        from concourse import bass2jax
        from jax.sharding import Mesh, NamedSharding, PartitionSpec
        from jax.experimental.shard_map import shard_map

        bass2jax.install_neuronx_cc_hook()
        self.jax = jax
        self.jnp = jnp
        in_names: list[str] = []
        out_names: list[str] = []
        out_avals = []
        out_shapes = []
        partition_name = (
            nc.partition_id_tensor.name if nc.partition_id_tensor else None
        )
        for alloc in nc.m.functions[0].allocations:
            if not isinstance(alloc, mybir.MemoryLocationSet):
                continue
            name = alloc.memorylocations[0].name
            if alloc.kind == "ExternalInput":
                if name != partition_name:
                    in_names.append(name)
            elif alloc.kind == "ExternalOutput":
                shape = tuple(alloc.tensor_shape)
                dtype = mybir.dt.np(alloc.dtype)
                out_names.append(name)
                out_avals.append(jax.core.ShapedArray(shape, dtype))
                out_shapes.append((shape, dtype))
        self.in_names = in_names
        self.out_names = out_names
        self.out_shapes = out_shapes
        n_params = len(in_names)
        n_outs = len(out_names)
        all_names = in_names + out_names
        if partition_name is not None:
            all_names = all_names + [partition_name]

        def _body(*args):
            operands = list(args)
            if partition_name is not None:
                operands.append(bass2jax.partition_id_tensor())
            outs = bass2jax._bass_exec_p.bind(
                *operands,
                out_avals=tuple(out_avals),
                in_names=tuple(all_names),
                out_names=tuple(out_names),
                lowering_input_output_aliases=(),
                sim_require_finite=True,
                sim_require_nnan=True,
                nc=nc,
            )
            return tuple(outs)

        devices = jax.devices()[:NCORES]
        self.mesh = Mesh(np.asarray(devices), ("core",))
        spec = PartitionSpec("core")
        self.sharding = NamedSharding(self.mesh, spec)
        self.sharded = jax.jit(
            shard_map(
                _body,
                mesh=self.mesh,
                in_specs=(spec,) * (n_params + n_outs),
                out_specs=(spec,) * n_outs,
                check_rep=False,
            ),
            donate_argnums=tuple(range(n_params, n_params + n_outs)),
            keep_unused=True,
        )

        def _mk_zeros():
            return tuple(
                jnp.zeros((NCORES * s[0], *s[1:]), d) for s, d in out_shapes
            )

        self.zeros_fn = jax.jit(
            _mk_zeros, out_shardings=(self.sharding,) * n_outs
        )

    def _stage_inputs(self, in_maps):
        concat = [
            np.concatenate(
                [np.asarray(in_maps[c][n]) for c in range(NCORES)], axis=0
            )
            for n in self.in_names
        ]
        return [self.jax.device_put(a, self.sharding) for a in concat]

    def _dispatch(self, staged):
        zeros = self.zeros_fn()
        outs = self.sharded(*staged, *zeros)
        self.jax.block_until_ready(outs)
        return outs

    def run(self, in_maps):
        staged = self._stage_inputs(in_maps)
        outs = self._dispatch(staged)
        res = []
        for c in range(NCORES):
            m = {}
            for i, n in enumerate(self.out_names):
                s, d = self.out_shapes[i]
                m[n] = np.asarray(outs[i]).reshape(NCORES, *s)[c]
            res.append(m)
        return res

    def timed(self, in_maps, iters=10):
        import time

        staged = self._stage_inputs(in_maps)
        self._dispatch(staged)  # warm
        walls = []
        for _ in range(iters):
            zeros = self.zeros_fn()
            self.jax.block_until_ready(zeros)
            t0 = time.perf_counter()
            outs = self.sharded(*staged, *zeros)
            self.jax.block_until_ready(outs)
            walls.append(time.perf_counter() - t0)
        tiny = self.jax.device_put(
            np.zeros((NCORES, 8), np.float32), self.sharding
        )
        base_fn = self.jax.jit(lambda a: a + 1.0)
        self.jax.block_until_ready(base_fn(tiny))
        bases = []
        for _ in range(iters):
            t0 = time.perf_counter()
            self.jax.block_until_ready(base_fn(tiny))
            bases.append(time.perf_counter() - t0)
        print(
            f"kernel walls min/med: {min(walls)*1e3:.2f}/"
            f"{np.median(walls)*1e3:.2f} ms; "
            f"baseline min/med: {min(bases)*1e3:.2f}/"
            f"{np.median(bases)*1e3:.2f} ms"
        )
        return max(float(np.median(walls) - np.median(bases)), 0.0) * 1e9


def _get_runner(nc):
    r = _RUNNER_CACHE.get(id(nc))
    if r is None:
        r = _PjrtRunner(nc)
        _RUNNER_CACHE[id(nc)] = r
    return r


def time_kernel(inputs=None, iters=8):
    assert _LAST is not None, "call kernel() first"
    nc, in_maps = _LAST
    return _get_runner(nc).timed(in_maps, iters=iters)


# revision 25
# speedup vs baseline: 8.5053x; 1.0198x over previous
"""Trainium2 Bass kernel for NodeNetworkG GNN message passing.

Algorithm (8 NeuronCores, SPMD, no collectives):
  - Nodes are sorted by total degree and dealt round-robin to 8 cores; each
    core owns ~1/8 of nodes and edges for both aggregation directions.
  - Per direction, destinations are packed into 128-node blocks (sorted by
    that direction's degree). Each edge is assigned to one of 8 "classes"
    (4 node-id range chunks + 4 mod-4 decimated copies) via 2-choice load
    balancing, so every gather instruction's int16 indices address a
    25088-row fp16 table (dma_gather is limited to 1024 indices/instruction).
  - Bulk tier: per (block, class) band with T slots per destination (T chosen
    per band to trade slot padding vs overflow, quantized over block groups).
    Batched dma_gather (96B rows from 256B-stride tables), DVE weight-multiply
    and uniform-T fold in fp16, accumulated into [128, nb*48] accumulators.
  - Overflow tier (edges beyond T): class-grouped dma_gather at full packing,
    weight-multiply, then dma_scatter_add into a DRAM accumulator (rank-sliced
    windows keep destinations unique per scatter instruction).
  - mo is realigned to mi-local node order via the DRAM accumulator and one
    dma_gather. Node-wise MLP per 128-node block: PE transpose to
    feature-major, two fp16 matmuls with tanh (ACT bias), written
    feature-major; host restores order.
"""

import numpy as np

P = 128
NCORES = 8
DIN = 48
DHID = 128
NPAD = 100352
NLOC = NPAD // NCORES      # 12544
NB = NLOC // P             # 98
CHUNK = NPAD // 4          # 25088
NCLS = 8
GMAX = 1024                # dma_gather / dma_scatter_add index limit
GCOLS = GMAX // P          # 8 columns per gather instruction
TILE_COLS = 32             # bulk G-tile columns (4 gather instrs)
NRANGE = 4                 # mi accumulator ranges for MLP pipelining
QUANT_LAMBDA = 2           # DP grouping: column-cost of an extra fold group
COST_SLOT = 1.45            # rel cost of one bulk slot (Pool+DMA+DVE)
COST_OVF = 4.5             # rel cost of one overflow edge

_PROG_CACHE: dict = {}


# --------------------------------------------------------------------------
# host prep
# --------------------------------------------------------------------------

def _class_assign(dst, src):
    """2-choice greedy per-destination class balancing. Returns cls [E]."""
    E = len(dst)
    cA = src // CHUNK
    cB = 4 + (src % 4)
    sortp = np.argsort(dst, kind="stable")
    degd = np.bincount(dst, minlength=NPAD)
    first = np.cumsum(degd) - degd
    srank_d = np.arange(E) - first[dst[sortp]]
    loads = np.zeros((NPAD, NCLS), np.int32)
    cls = np.empty(E, np.int64)
    maxr = int(srank_d.max()) if E else 0
    for r in range(maxr + 1):
        sel = sortp[srank_d == r]
        if len(sel) == 0:
            continue
        d = dst[sel]
        la = loads[d, cA[sel]]
        lb = loads[d, cB[sel]]
        pick_b = lb < la
        c = np.where(pick_b, cB[sel], cA[sel])
        loads[d, c] += 1
        cls[sel] = c
    return cls


def _quantize_groups(T_col, cuts):
    """DP: partition blocks into contiguous groups (respecting forced cuts),
    pad T to group max. Returns [(b0, nbs, Tq)] with Tq>0 only."""
    groups = []
    for ci in range(len(cuts) - 1):
        lo, hi = cuts[ci], cuts[ci + 1]
        n = hi - lo
        if n <= 0:
            continue
        seg = T_col[lo:hi]
        INF = 1 << 60
        best = [INF] * (n + 1)
        prev = [0] * (n + 1)
        best[0] = 0
        for j in range(1, n + 1):
            m = 0
            ssum = 0
            for i in range(j - 1, -1, -1):
                if seg[i] > m:
                    m = seg[i]
                ssum += seg[i]
                cost = best[i] + (m * (j - i) - ssum) + QUANT_LAMBDA
                if cost < best[j]:
                    best[j] = cost
                    prev[j] = i
        bounds = []
        j = n
        while j > 0:
            bounds.append((prev[j], j))
            j = prev[j]
        for i, j in reversed(bounds):
            Tq = int(seg[i:j].max())
            if Tq > 0:
                groups.append((lo + i, j - i, Tq))
    return groups


def _wrap_idx(vals16, pos):
    """Scatter int16 values into the wrapped-by-16, 8x-replicated layout.
    vals16 [n], pos [n] list positions. Returns writes for a [16, L] array:
    (rows, cols)."""
    return (pos % 16).astype(np.int64), pos // 16


def _build_direction(dst, src, w, deg, core, ranges):
    """Full per-direction layout: bulk bands + overflow lists."""
    loc = np.empty(NPAD, np.int64)
    nodes_by_core = []
    for k in range(NCORES):
        nodes_k = np.where(core == k)[0]
        lk = nodes_k[np.argsort(-deg[nodes_k], kind="stable")]
        loc[lk] = np.arange(NLOC)
        nodes_by_core.append(lk)
    blk = loc // P
    part = loc % P

    cls = _class_assign(dst, src)
    E = len(dst)

    # per-edge rank within (dest, class)
    key = dst * NCLS + cls
    sortp = np.argsort(key, kind="stable")
    ks = key[sortp]
    cnt = np.bincount(ks, minlength=NPAD * NCLS)
    firstk = np.cumsum(cnt) - cnt
    srank = np.empty(E, np.int64)
    srank[sortp] = np.arange(E) - firstk[ks]

    loads = cnt.reshape(NPAD, NCLS)

    # T* per (block, class): minimize 128*T*COST_SLOT + unified_ovf(T)*COST_OVF
    maxl = int(loads.max())
    # histogram of per-(core, block, class) dest-loads
    hist = np.zeros((NCORES, NB, NCLS, maxl + 1), np.int64)
    nodes = np.arange(NPAD)
    for c in range(NCLS):
        np.add.at(hist, (core[nodes], blk[nodes], c, loads[nodes, c]), 1)
    # abovec[..., lv] = #dests with load > lv ; ovf(T) = sum_{lv>=T} abovec[lv]
    cum = hist.cumsum(axis=3)
    total = cum[..., -1:]
    abovec = (total - cum)  # [..., lv] = #dests with load > lv
    suf = abovec[..., ::-1].cumsum(axis=3)[..., ::-1]  # per-core ovf at T=lv
    ovf_unified = suf.max(axis=0)  # [NB, NCLS, maxl+1]
    Tcost = (128 * np.arange(maxl + 1)[None, None, :] * COST_SLOT
             + ovf_unified * COST_OVF)
    Tstar = Tcost.argmin(axis=2)  # [NB, NCLS]

    cuts = sorted(set([0, NB] + list(ranges)))
    groups_per_cls = [_quantize_groups(Tstar[:, c], cuts) for c in range(NCLS)]
    Tq = np.zeros((NB, NCLS), np.int64)
    for c in range(NCLS):
        for b0, nbs, t in groups_per_cls[c]:
            Tq[b0 : b0 + nbs, c] = t

    # ---- bulk tile packing ----
    # Per class: one continuous column run over all its groups (block order),
    # split into tiles of TILE_COLS; gather windows stay full except at each
    # class's tail. Tiles are interleaved across classes by block progress so
    # mi ranges complete roughly in order.
    cls_tiles = {c: [] for c in range(NCLS)}
    colstart = np.full((NB, NCLS), -1, np.int64)
    col = 0
    for c in range(NCLS):
        runs = []
        for b0, nbs, t in groups_per_cls[c]:
            nmax = max(1, TILE_COLS // t)
            j = 0
            while j < nbs:
                nn = min(nmax, nbs - j)
                runs.append((b0 + j, nn, t))
                j += nn
        cur = None
        for b0, nbs, t in runs:
            need = nbs * t
            if cur is None or cur["cols"] + need > TILE_COLS:
                if cur is not None:
                    cls_tiles[c].append(cur)
                cur = dict(cls=c, col0=col, cols=0, entries=[])
            cur["entries"].append((t, b0, nbs, cur["cols"]))
            for i in range(nbs):
                colstart[b0 + i, c] = col + i * t
            cur["cols"] += need
            col += need
        if cur is not None:
            cls_tiles[c].append(cur)
    CT = col
    # interleave tiles by first-block progress
    tiles = sorted(
        (t for c in range(NCLS) for t in cls_tiles[c]),
        key=lambda t: (t["entries"][0][1], t["cls"]),
    )

    percol_col0 = np.zeros(max(CT, 1), np.int64)
    for t in tiles:
        percol_col0[t["col0"] : t["col0"] + t["cols"]] = t["col0"]

    is_bulk = srank < Tq[blk[dst], cls]

    # class-local index values
    val = np.where(cls < 4, src - (cls * CHUNK), src // 4).astype(np.int16)

    eb = np.where(is_bulk)[0]
    d = dst[eb]
    k_e = core[d]
    b_e = blk[d]
    p_e = part[d]
    colg = colstart[b_e, cls[eb]] + srank[eb]
    col0 = percol_col0[colg]
    g_t = (colg - col0) * P + p_e
    ic = col0 * 8 + g_t // 16
    ir = g_t % 16

    idx16 = np.zeros((NCORES, 16, max(CT, 1) * 8), np.int16)
    idx16[k_e, ir, ic] = val[eb]
    idx_arr = np.tile(idx16, (1, 8, 1))
    w_arr = np.zeros((NCORES, P, max(CT, 1)), np.float16)
    w_arr[k_e, p_e, colg] = w[eb].astype(np.float16)

    # ---- overflow lists ----
    # order: (rank', class, dest); windows of GMAX cut at rank boundaries
    eo = np.where(~is_bulk)[0]
    # global per-dest overflow rank (across classes): scatter windows slice by
    # rank, so a dest must appear at most once per rank block
    osort0 = np.argsort(dst[eo], kind="stable")
    dso = dst[eo][osort0]
    dcnt = np.bincount(dso, minlength=NPAD)
    dfirst = np.cumsum(dcnt) - dcnt
    rank2 = np.empty(len(eo), np.int64)
    rank2[osort0] = np.arange(len(eo)) - dfirst[dso]
    # rotate each dest's rank slots (stays a permutation per dest) so the
    # per-(rank, class) segment sizes balance across cores
    kd = dcnt[dst[eo]]
    rank2 = np.where(kd > 0, (rank2 + dst[eo]) % np.maximum(kd, 1), rank2)
    # per-core list; unified layout = same windows/segments across cores, so
    # build per-core orderings with shared per-(rank, class) segment sizes
    segcnt = np.zeros((NCORES, int(rank2.max()) + 1 if len(eo) else 1, NCLS),
                      np.int64)
    if len(eo):
        np.add.at(segcnt, (core[dst[eo]], rank2, cls[eo]), 1)
    useg = segcnt.max(axis=0)  # unified per-(rank, class) segment length
    # pad each segment to a 128 multiple: every gather instruction restarts
    # its list at partition 0, so segments must start 128-aligned
    useg = -(-useg // P) * P
    NRK = useg.shape[0]
    seg_off = np.zeros((NRK, NCLS), np.int64)
    posn = 0
    ov_windows = []   # pos0 per GMAX window
    ov_gathers = []   # (cls, pos0, n) per gather instruction (<= GMAX, class-pure)
    for r in range(NRK):
        for c in range(NCLS):
            n = int(useg[r, c])
            if n == 0:
                continue
            seg_off[r, c] = posn
            s = 0
            while s < n:
                woff = (posn + s) % GMAX
                take = min(n - s, GMAX - woff)
                ov_gathers.append((c, posn + s, take))
                s += take
            posn += n
        posn = -(-posn // GMAX) * GMAX  # pad rank block to window multiple
    OVT = posn // P  # overflow columns
    for wpos in range(0, posn, GMAX):
        ov_windows.append(wpos)

    ov_idx16 = np.zeros((NCORES, 16, max(OVT, 1) * 8), np.int16)
    ov_sidx16 = np.full((NCORES, 16, max(OVT, 1) * 8), NLOC, np.int16)
    ov_w = np.zeros((NCORES, P, max(OVT, 1)), np.float16)
    if len(eo):
        # per-core position within segment: stable order by (core,rank,cls)
        okey = (core[dst[eo]] * NRK + rank2) * NCLS + cls[eo]
        osort = np.argsort(okey, kind="stable")
        oks = okey[osort]
        ocnt = np.bincount(oks, minlength=NCORES * NRK * NCLS)
        ofirst = np.cumsum(ocnt) - ocnt
        opos = np.empty(len(eo), np.int64)
        opos[osort] = np.arange(len(eo)) - ofirst[oks]
        e = eo
        posg = seg_off[rank2, cls[e]] + opos  # list position
        kk = core[dst[e]]
        ov_idx16[kk, posg % 16, (posg // 16)] = val[e]
        ov_sidx16[kk, posg % 16, (posg // 16)] = loc[dst[e]].astype(np.int16)
        ov_w[kk, posg % P, posg // P] = w[e].astype(np.float16)
    ov_idx = np.tile(ov_idx16, (1, 8, 1))
    ov_sidx = np.tile(ov_sidx16, (1, 8, 1))

    return dict(
        loc=loc, blk=blk, part=part, nodes_by_core=nodes_by_core,
        tiles=tiles, CT=CT, idx_arr=idx_arr, w_arr=w_arr, Tq=Tq,
        OVT=OVT, ov_idx=ov_idx, ov_sidx=ov_sidx, ov_w=ov_w,
        ov_windows=ov_windows, ov_gathers=ov_gathers,
    )


def _host_prep(x, edge_index, edge_attr):
    N = x.shape[0]
    row = np.asarray(edge_index[0]).astype(np.int64)
    col = np.asarray(edge_index[1]).astype(np.int64)
    w = np.asarray(edge_attr, dtype=np.float32).reshape(-1)

    deg_in = np.bincount(col, minlength=NPAD)
    deg_out = np.bincount(row, minlength=NPAD)
    order = np.argsort(-(deg_in + deg_out), kind="stable")
    rank = np.empty(NPAD, np.int64)
    rank[order] = np.arange(NPAD)
    core = (rank % NCORES).astype(np.int64)

    ranges = [32, 64, 94]

    dmi = _build_direction(col, row, w, deg_in, core, ranges)
    dmo = _build_direction(row, col, w, deg_out, core, [])

    xf = np.zeros((NPAD, DIN), np.float32)
    xf[:N] = np.asarray(x, np.float32)
    x16 = xf.astype(np.float16)
    tabs = np.zeros((NCLS, CHUNK, 128), np.float16)
    for c in range(4):
        tabs[c, :, :DIN] = x16[c * CHUNK : (c + 1) * CHUNK]
    for r in range(4):
        tabs[4 + r, :, :DIN] = x16[r::4]

    x_own = np.zeros((NCORES, P, NB * DIN), np.float16)
    for k in range(NCORES):
        lk = dmi["nodes_by_core"][k]
        xv = x16[lk]
        x_own[k] = (
            xv.reshape(NB, P, DIN).transpose(1, 0, 2).reshape(P, NB * DIN)
        )

    realign = np.zeros((NCORES, 16, NLOC // 16), np.int16)
    g = np.arange(NLOC)
    for k in range(NCORES):
        lk = dmi["nodes_by_core"][k]
        vals = dmo["loc"][lk].astype(np.int16)
        realign[k, g % 16, g // 16] = vals
    realign = np.tile(realign, (1, 8, 1))

    return dict(
        N=N, core=core, dmi=dmi, dmo=dmo, tabs=tabs, x_own=x_own,
        realign=realign,
    )


# --------------------------------------------------------------------------
# numpy emulation (layout validation)
# --------------------------------------------------------------------------

def _emulate_agg(meta, direction):
    """Emulate both tiers -> acc [NCORES, 128, NB*48] fp32 in local order."""
    d = meta[direction]
    tabs = meta["tabs"]
    CT = d["CT"]
    acc = np.zeros((NCORES, P, NB, DIN), np.float32)
    colcls = np.zeros(max(CT, 1), np.int64)
    colblk = np.zeros(max(CT, 1), np.int64)
    col0a = np.zeros(max(CT, 1), np.int64)
    for t in d["tiles"]:
        col0a[t["col0"] : t["col0"] + t["cols"]] = t["col0"]
        for s, b0, nbs, cbase in t["entries"]:
            for i in range(nbs):
                c0 = t["col0"] + cbase + i * s
                colcls[c0 : c0 + s] = t["cls"]
                colblk[c0 : c0 + s] = b0 + i
    for k in range(NCORES):
        if CT:
            idx = d["idx_arr"][k]
            wv = d["w_arr"][k]
            cols = np.arange(CT)
            pp = np.arange(P)
            g_t = (cols[None, :] - col0a[None, :]) * P + pp[:, None]
            ic = col0a[None, :] * 8 + g_t // 16
            ir = g_t % 16
            vals = idx[ir, ic].astype(np.int64)
            gathered = tabs[colcls[None, :].repeat(P, 0), vals][:, :, :DIN]
            prod = gathered.astype(np.float16) * wv[:, :, None]
            np.add.at(acc[k], (slice(None), colblk), prod.astype(np.float32))
        # overflow
        OVT = d["OVT"]
        if OVT:
            oi = d["ov_idx"][k]
            os_ = d["ov_sidx"][k]
            ow = d["ov_w"][k]
            # reconstruct per-position
            ocls = np.zeros(OVT * P, np.int64)
            for c, pos0, n in d["ov_gathers"]:
                ocls[pos0 : pos0 + n] = c
            g = np.arange(OVT * P)
            vals = oi[g % 16, g // 16].astype(np.int64)
            sidx = os_[g % 16, g // 16].astype(np.int64)
            wvals = ow[g % P, g // P].astype(np.float16)
            gath = tabs[ocls, vals][:, :DIN].astype(np.float16)
            prod = (gath * wvals[:, None]).astype(np.float32)
            sel = sidx < NLOC
            tgt = sidx[sel]
            np.add.at(acc[k], (tgt % P, tgt // P), prod[sel])
    return acc.reshape(NCORES, P, NB * DIN)


def _emulate(meta, W1, b1, W2, b2):
    mi = _emulate_agg(meta, "dmi")
    mo = _emulate_agg(meta, "dmo")
    out = np.zeros((NPAD, DHID), np.float32)
    for k in range(NCORES):
        lk_i = meta["dmi"]["nodes_by_core"][k]
        lk_o = meta["dmo"]["nodes_by_core"][k]
        mi_k = mi[k].reshape(P, NB, DIN).transpose(1, 0, 2).reshape(NLOC, DIN)
        mo_k = mo[k].reshape(P, NB, DIN).transpose(1, 0, 2).reshape(NLOC, DIN)
        mo_full = np.zeros((NLOC, DIN), np.float32)
        mo_full[:] = mo_k  # mo-local order
        mo2_k = mo_full[meta["dmo"]["loc"][lk_i]]
        x_k = meta["x_own"][k].reshape(P, NB, DIN).transpose(1, 0, 2).reshape(
            NLOC, DIN
        ).astype(np.float32)
        M = np.concatenate([mi_k, mo2_k, x_k], axis=1)
        h = np.tanh(M @ W1.T + b1)
        out[lk_i] = np.tanh(h @ W2.T + b2)
    return out[: meta["N"]]


# --------------------------------------------------------------------------
# device program
# --------------------------------------------------------------------------

def _dma_gather96(gp, mybir, out_ap, in_ap, idxs_ap, num_idxs):
    from concourse.bass import exact_div

    elem_step = in_ap.ap[0][0]
    stride_bytes = elem_step * mybir.dt.size(in_ap.dtype)
    return gp.add_instruction(
        mybir.InstDMAGatherAnt(
            name=gp.bass.get_next_instruction_name(),
            ins=[
                *gp.lower_ap_dma(in_ap, for_custom_bir_dma=True),
                gp.lower_ap(idxs_ap),
                gp.lower_val_access(gp.to_reg(num_idxs)),
            ],
            outs=[gp.lower_ap(out_ap)],
            transpose=False,
            num_idxs=num_idxs,
            elem_size=DIN,
            stride_bytes_256=exact_div(stride_bytes, 256),
            gen_mode=0,
            single_packet=True,
            queue_num=0,
            sbuf_tokens_per_rank=0,
            sbuf_free_dim_per_rank=0,
            sbuf_free_dim_pad_per_rank=0,
            sbuf_byte_offset=0,
        )
    )


def _build_program(meta):
    import concourse.bacc as bacc
    import concourse.bass as bass
    import concourse.mybir as mybir
    import concourse.tile as tile
    from concourse.masks import make_identity

    f32 = mybir.dt.float32
    f16 = mybir.dt.float16
    i16 = mybir.dt.int16

    dmi, dmo = meta["dmi"], meta["dmo"]
    CTI, CTO = max(dmi["CT"], 1), max(dmo["CT"], 1)
    OVI, OVO = max(dmi["OVT"], 1), max(dmo["OVT"], 1)

    nc = bacc.Bacc(
        "TRN2",
        target_bir_lowering=False,
        debug=False,
        num_devices=NCORES,
        dynamic_dma_scratch_size=65536,
    )

    tabs_d = [
        nc.dram_tensor(f"tab{c}", [CHUNK, 128], f16, kind="ExternalInput")
        for c in range(NCLS)
    ]
    idx_mi_d = nc.dram_tensor("idx_mi", [P, CTI * 8], i16, kind="ExternalInput")
    idx_mo_d = nc.dram_tensor("idx_mo", [P, CTO * 8], i16, kind="ExternalInput")
    w_mi_d = nc.dram_tensor("w_mi", [P, CTI], f16, kind="ExternalInput")
    w_mo_d = nc.dram_tensor("w_mo", [P, CTO], f16, kind="ExternalInput")
    ovi_idx_d = nc.dram_tensor("ovi_idx", [P, OVI * 8], i16, kind="ExternalInput")
    ovi_sidx_d = nc.dram_tensor("ovi_sidx", [P, OVI * 8], i16, kind="ExternalInput")
    ovi_w_d = nc.dram_tensor("ovi_w", [P, OVI], f16, kind="ExternalInput")
    ovo_idx_d = nc.dram_tensor("ovo_idx", [P, OVO * 8], i16, kind="ExternalInput")
    ovo_sidx_d = nc.dram_tensor("ovo_sidx", [P, OVO * 8], i16, kind="ExternalInput")
    ovo_w_d = nc.dram_tensor("ovo_w", [P, OVO], f16, kind="ExternalInput")
    x_own_d = nc.dram_tensor("x_own", [P, NB * DIN], f16, kind="ExternalInput")
    realign_d = nc.dram_tensor("realign", [P, NLOC // 16], i16, kind="ExternalInput")
    mi_acc = nc.dram_tensor("mi_acc", [NLOC + P, 128], f16, kind="Internal")
    mo_acc = nc.dram_tensor("mo_acc", [NLOC + P, 128], f16, kind="Internal")
    w1ta_d = nc.dram_tensor("w1ta", [DIN, DHID], f16, kind="ExternalInput")
    w1tb_d = nc.dram_tensor("w1tb", [DIN, DHID], f16, kind="ExternalInput")
    w1tc_d = nc.dram_tensor("w1tc", [DIN, DHID], f16, kind="ExternalInput")
    w2t_d = nc.dram_tensor("w2t", [DHID, DHID], f16, kind="ExternalInput")
    b1_d = nc.dram_tensor("b1", [DHID, 1], f32, kind="ExternalInput")
    b2_d = nc.dram_tensor("b2", [DHID, 1], f32, kind="ExternalInput")
    out_t = nc.dram_tensor("out_t", [P, NLOC], f16, kind="ExternalOutput")

    rbounds = [0, 32, 64, 94, NB]

    with tile.TileContext(nc) as tc:
        with (
            tc.tile_pool(name="const", bufs=1) as const,
            tc.tile_pool(name="gidx", bufs=6) as gidx,
            tc.tile_pool(name="gpool", bufs=6) as gpool,
            tc.tile_pool(name="mlp", bufs=4) as mlp,
            tc.tile_pool(name="ost", bufs=2) as ostp,
            tc.tile_pool(name="psT", bufs=3, space="PSUM") as psT,
            tc.tile_pool(name="psH", bufs=2, space="PSUM") as psH,
        ):
            w_mi_sb = const.tile([P, CTI], f16)
            nc.scalar.dma_start(w_mi_sb[:], w_mi_d[:])
            w_mo_sb = const.tile([P, CTO], f16)
            nc.scalar.dma_start(w_mo_sb[:], w_mo_d[:])
            ovi_w_sb = const.tile([P, OVI], f16)
            nc.scalar.dma_start(ovi_w_sb[:], ovi_w_d[:])
            ovo_w_sb = const.tile([P, OVO], f16)
            nc.scalar.dma_start(ovo_w_sb[:], ovo_w_d[:])
            realign_sb = const.tile([P, NLOC // 16], i16)
            nc.sync.dma_start(realign_sb[:], realign_d[:])
            w1ta_sb = const.tile([DIN, DHID], f16)
            nc.sync.dma_start(w1ta_sb[:], w1ta_d[:])
            w1tb_sb = const.tile([DIN, DHID], f16)
            nc.sync.dma_start(w1tb_sb[:], w1tb_d[:])
            w1tc_sb = const.tile([DIN, DHID], f16)
            nc.sync.dma_start(w1tc_sb[:], w1tc_d[:])
            w2t_sb = const.tile([DHID, DHID], f16)
            nc.sync.dma_start(w2t_sb[:], w2t_d[:])
            b1_sb = const.tile([DHID, 1], f32)
            nc.sync.dma_start(b1_sb[:], b1_d[:])
            b2_sb = const.tile([DHID, 1], f32)
            nc.sync.dma_start(b2_sb[:], b2_d[:])
            ident = const.tile([P, P], f16)
            make_identity(nc, ident[:])
            xo_sb = const.tile([P, NB * DIN], f16)
            nc.scalar.dma_start(xo_sb[:], x_own_d[:])

            mo_sb = const.tile([P, NB * DIN], f16)
            mo2_sb = const.tile([P, NB * DIN], f16)
            mi_rs = []
            for ri in range(NRANGE):
                mi_ri = const.tile([P, (rbounds[ri + 1] - rbounds[ri]) * DIN],
                                   f16, name=f"mi_r{ri}")
                mi_rs.append(mi_ri)
                nc.vector.memset(mi_ri[:], 0.0)
            zz = const.tile([P, (NB + 1) * DIN], f16)
            nc.vector.memset(mo_sb[:], 0.0)
            nc.vector.memset(zz[:], 0.0)
            # zero DRAM accumulators (NLOC + dummy block rows)
            for acc_d in (mi_acc, mo_acc):
                nc.scalar.dma_start(
                    acc_d[:, 0:DIN].rearrange("(b p) f -> p b f", p=P),
                    zz[:].rearrange("p (b f) -> p b f", f=DIN),
                )

            def emit_interleaved(dirmeta, idx_d, w_sb, acc_of,
                                 oidx_d, osidx_d, ow_sb, acc_d,
                                 only_range=None):
                tiles = [t for t in dirmeta["tiles"]
                         if only_range is None
                         or only_range[0] <= t["entries"][0][1] < only_range[1]]
                wins = list(dirmeta["ov_windows"]) if only_range is None or \
                    only_range[0] == 0 else []
                step = max(1, (len(tiles) + len(wins)) // max(len(wins), 1)) \
                    if wins else 1 << 30
                wi = 0
                for i, t in enumerate(tiles):
                    emit_one_bulk(dirmeta, t, idx_d, w_sb, acc_of)
                    if wi < len(wins) and i % step == step - 1:
                        emit_overflow(dirmeta, oidx_d, osidx_d, ow_sb, acc_d,
                                      [wins[wi]])
                        wi += 1
                if wi < len(wins):
                    emit_overflow(dirmeta, oidx_d, osidx_d, ow_sb, acc_d,
                                  wins[wi:])

            def emit_bulk(dirmeta, idx_d, w_sb, acc_of, only_range=None):
                rlo, rhi = (0, NB) if only_range is None else only_range
                for t in dirmeta["tiles"]:
                    b_first = t["entries"][0][1]
                    if not (rlo <= b_first < rhi):
                        continue
                    emit_one_bulk(dirmeta, t, idx_d, w_sb, acc_of)

            def emit_one_bulk(dirmeta, t, idx_d, w_sb, acc_of):
                    cols = t["cols"]
                    c = t["cls"]
                    col0 = t["col0"]
                    idx_sb = gidx.tile([P, TILE_COLS * 8], i16, tag="gi")
                    nc.sync.dma_start(
                        idx_sb[:, : cols * 8],
                        idx_d[:, col0 * 8 : (col0 + cols) * 8],
                    )
                    G = gpool.tile([P, TILE_COLS * DIN], f16, tag="G")
                    for q0 in range(0, cols, GCOLS):
                        qn = min(GCOLS, cols - q0)
                        _dma_gather96(
                            nc.gpsimd, mybir,
                            out_ap=G[:, q0 * DIN : (q0 + qn) * DIN].rearrange(
                                "p (c f) -> p c f", f=DIN
                            ),
                            in_ap=tabs_d[c][:, 0:DIN],
                            idxs_ap=idx_sb[:, q0 * 8 : (q0 + qn) * 8],
                            num_idxs=qn * P,
                        )
                    g3 = G[:, : cols * DIN].rearrange("p (c f) -> p c f", f=DIN)
                    wv = w_sb[:, col0 : col0 + cols]
                    wb = bass.AP(
                        wv.tensor,
                        wv.offset,
                        [list(wv.ap[0]), list(wv.ap[1]), [0, DIN]],
                    )
                    nc.vector.tensor_tensor(
                        out=g3, in0=g3, in1=wb, op=mybir.AluOpType.mult
                    )
                    for s, b0, nbs, cbase in t["entries"]:
                        gg = G[
                            :, cbase * DIN : (cbase + nbs * s) * DIN
                        ].rearrange("p (b s f) -> p b s f", s=s, f=DIN)
                        ss = s
                        while ss > 1:
                            half = ss // 2
                            hi0 = ss - half
                            nc.vector.tensor_tensor(
                                out=gg[:, :, 0:half, :],
                                in0=gg[:, :, 0:half, :],
                                in1=gg[:, :, hi0:ss, :],
                                op=mybir.AluOpType.add,
                            )
                            ss = hi0
                        acc_sb, boff = acc_of(b0)
                        accv = acc_sb[
                            :, (b0 - boff) * DIN : (b0 - boff + nbs) * DIN
                        ].rearrange("p (b f) -> p b f", f=DIN)
                        nc.vector.tensor_tensor(
                            out=accv, in0=accv, in1=gg[:, :, 0, :],
                            op=mybir.AluOpType.add,
                        )

            def emit_overflow(dirmeta, oidx_d, osidx_d, ow_sb, acc_d,
                              subset=None):
                gathers = dirmeta["ov_gathers"]
                windows = dirmeta["ov_windows"] if subset is None else subset
                for wpos in windows:
                    gi = 0
                    while gi < len(gathers) and gathers[gi][1] < wpos:
                        gi += 1
                    idx_sb = gidx.tile([P, GCOLS * 8], i16, tag="oi")
                    nc.sync.dma_start(
                        idx_sb[:],
                        oidx_d[:, (wpos // 16) : (wpos // 16) + GCOLS * 8],
                    )
                    sidx_sb = gidx.tile([P, GCOLS * 8], i16, tag="os")
                    nc.sync.dma_start(
                        sidx_sb[:],
                        osidx_d[:, (wpos // 16) : (wpos // 16) + GCOLS * 8],
                    )
                    G = gpool.tile([P, GCOLS * DIN], f16, tag="G")
                    while gi < len(gathers) and gathers[gi][1] < wpos + GMAX:
                        c, pos0, n = gathers[gi]
                        lo = pos0 - wpos
                        _dma_gather96(
                            nc.gpsimd, mybir,
                            out_ap=G[
                                :, (lo // P) * DIN : ((lo + n) // P) * DIN
                            ].rearrange("p (c f) -> p c f", f=DIN),
                            in_ap=tabs_d[c][:, 0:DIN],
                            idxs_ap=idx_sb[:, lo // 16 : (lo + n) // 16],
                            num_idxs=n,
                        )
                        gi += 1
                    g3 = G[:].rearrange("p (c f) -> p c f", f=DIN)
                    wv = ow_sb[:, wpos // P : wpos // P + GCOLS]
                    wb = bass.AP(
                        wv.tensor,
                        wv.offset,
                        [list(wv.ap[0]), list(wv.ap[1]), [0, DIN]],
                    )
                    nc.vector.tensor_tensor(
                        out=g3, in0=g3, in1=wb, op=mybir.AluOpType.mult
                    )
                    nc.gpsimd.dma_scatter_add(
                        out_ap=acc_d[:, 0:DIN],
                        in_ap=g3,
                        idxs_ap=sidx_sb[:],
                        num_idxs=GMAX,
                        num_idxs_reg=GMAX,
                        elem_size=DIN,
                        elem_step=128,
                    )

            def acc_mo(b0):
                return mo_sb, 0

            def acc_mi(b0):
                for ri in range(NRANGE):
                    if b0 < rbounds[ri + 1]:
                        return mi_rs[ri], rbounds[ri]
                raise AssertionError(b0)

            # ---- mo: bulk with overflow windows interleaved ----
            emit_interleaved(dmo, idx_mo_d, w_mo_sb, acc_mo,
                             ovo_idx_d, ovo_sidx_d, ovo_w_sb, mo_acc)
            # merge: mo_sb += mo_acc; write back
            tmp = const.tile([P, NB * DIN], f16)
            nc.sync.dma_start(
                tmp[:].rearrange("p (b f) -> p b f", f=DIN),
                mo_acc[0:NLOC, 0:DIN].rearrange("(b p) f -> p b f", p=P),
            )
            nc.vector.tensor_tensor(
                out=mo_sb[:], in0=mo_sb[:], in1=tmp[:], op=mybir.AluOpType.add
            )
            nc.sync.dma_start(
                mo_acc[0:NLOC, 0:DIN].rearrange("(b p) f -> p b f", p=P),
                mo_sb[:].rearrange("p (b f) -> p b f", f=DIN),
            )

            def emit_mlp(rlo, rhi):
                OG = 4
                for b0 in range(rlo, rhi, OG):
                    og = min(OG, rhi - b0)
                    os_ = ostp.tile([P, OG * P], f16, tag="oo")
                    for j in range(og):
                        b = b0 + j
                        hp = psH.tile([P, P], f32, tag="hp")
                        mi_t, mi_b0 = acc_mi(b)
                        for q, (src_sb, bb, w1q) in enumerate((
                            (mi_t, b - mi_b0, w1ta_sb),
                            (mo2_sb, b, w1tb_sb),
                            (xo_sb, b, w1tc_sb),
                        )):
                            pA = psT.tile([DIN, P], f16, tag="pA")
                            nc.tensor.transpose(
                                pA[:], src_sb[:, bb * DIN : (bb + 1) * DIN],
                                ident[:],
                            )
                            mt = mlp.tile([DIN, P], f16, tag="mt")
                            nc.scalar.copy(out=mt[:], in_=pA[:])
                            nc.tensor.matmul(
                                hp[:], w1q[:], mt[:],
                                start=(q == 0), stop=(q == 2),
                            )
                        hs = mlp.tile([P, P], f16, tag="hs")
                        nc.scalar.activation(
                            hs[:], hp[:],
                            mybir.ActivationFunctionType.Tanh,
                            bias=b1_sb[:], scale=1.0,
                        )
                        op_ = psH.tile([P, P], f32, tag="op")
                        nc.tensor.matmul(
                            op_[:], w2t_sb[:], hs[:], start=True, stop=True
                        )
                        nc.scalar.activation(
                            os_[:, j * P : (j + 1) * P], op_[:],
                            mybir.ActivationFunctionType.Tanh,
                            bias=b2_sb[:], scale=1.0,
                        )
                    nc.sync.dma_start(
                        out_t[:, b0 * P : (b0 + og) * P], os_[:, : og * P]
                    )

            realigned = False
            for ri in range(NRANGE):
                rlo, rhi = rbounds[ri], rbounds[ri + 1]
                if ri == 0:
                    emit_interleaved(dmi, idx_mi_d, w_mi_sb, acc_mi,
                                     ovi_idx_d, ovi_sidx_d, ovi_w_sb, mi_acc,
                                     (rlo, rhi))
                else:
                    emit_bulk(dmi, idx_mi_d, w_mi_sb, acc_mi, (rlo, rhi))
                if not realigned:
                    # realign mo to mi-local order (after some mi bulk so the
                    # Pool queue is not head-of-line blocked on the mo merge)
                    for g0 in range(0, NLOC, GMAX):
                        gn = min(GMAX, NLOC - g0)
                        _dma_gather96(
                            nc.gpsimd, mybir,
                            out_ap=mo2_sb[
                                :, (g0 // P) * DIN : ((g0 + gn) // P) * DIN
                            ].rearrange("p (b f) -> p b f", f=DIN),
                            in_ap=mo_acc[:, 0:DIN],
                            idxs_ap=realign_sb[:, g0 // 16 : (g0 + gn) // 16],
                            num_idxs=gn,
                        )
                    realigned = True
                # merge overflow acc for this range
                tmpr_full = gpool.tile([P, 32 * DIN], f16, tag="mr")
                tmpr = tmpr_full[:, : (rhi - rlo) * DIN]
                nc.sync.dma_start(
                    tmpr[:].rearrange("p (b f) -> p b f", f=DIN),
                    mi_acc[rlo * P : rhi * P, 0:DIN].rearrange(
                        "(b p) f -> p b f", p=P
                    ),
                )
                nc.vector.tensor_tensor(
                    out=mi_rs[ri][:],
                    in0=mi_rs[ri][:],
                    in1=tmpr[:],
                    op=mybir.AluOpType.add,
                )
                emit_mlp(rlo, rhi)

    nc.compile()
    return nc


# --------------------------------------------------------------------------
# entry point
# --------------------------------------------------------------------------

def kernel(x, edge_index, edge_attr, W1, b1, W2, b2):
    x = np.asarray(x, np.float32)
    meta = _host_prep(x, edge_index, edge_attr)
    dmi, dmo = meta["dmi"], meta["dmo"]
    key = (meta["N"], dmi["CT"], dmo["CT"], dmi["OVT"], dmo["OVT"],
           tuple(t["col0"] for t in dmi["tiles"]),
           tuple(t["col0"] for t in dmo["tiles"]),
           tuple(dmi["ov_gathers"]), tuple(dmo["ov_gathers"]))
    if key not in _PROG_CACHE:
        _PROG_CACHE[key] = _build_program(meta)
    nc = _PROG_CACHE[key]

    W1 = np.asarray(W1, np.float32)
    W2 = np.asarray(W2, np.float32)
    b1v = np.asarray(b1, np.float32).reshape(DHID, 1)
    b2v = np.asarray(b2, np.float32).reshape(DHID, 1)
    w1t = np.ascontiguousarray(W1.T)
    w1ta = np.ascontiguousarray(w1t[:DIN]).astype(np.float16)
    w1tb = np.ascontiguousarray(w1t[DIN : 2 * DIN]).astype(np.float16)
    w1tc = np.ascontiguousarray(w1t[2 * DIN :]).astype(np.float16)
    w2t = np.ascontiguousarray(W2.T).astype(np.float16)

    in_maps = []
    for k in range(NCORES):
        m = {
            "idx_mi": dmi["idx_arr"][k],
            "idx_mo": dmo["idx_arr"][k],
            "w_mi": dmi["w_arr"][k],
            "w_mo": dmo["w_arr"][k],
            "ovi_idx": dmi["ov_idx"][k],
            "ovi_sidx": dmi["ov_sidx"][k],
            "ovi_w": dmi["ov_w"][k],
            "ovo_idx": dmo["ov_idx"][k],
            "ovo_sidx": dmo["ov_sidx"][k],
            "ovo_w": dmo["ov_w"][k],
            "x_own": meta["x_own"][k],
            "realign": meta["realign"][k],
            "w1ta": w1ta, "w1tb": w1tb, "w1tc": w1tc, "w2t": w2t,
            "b1": b1v, "b2": b2v,
        }
        for c in range(NCLS):
            m[f"tab{c}"] = meta["tabs"][c]
        in_maps.append(m)

    runner = _get_runner(nc)
    results = runner.run(in_maps)
    global _LAST
    _LAST = (nc, in_maps)

    out = np.empty((NPAD, DHID), np.float32)
    for k in range(NCORES):
        out[dmi["nodes_by_core"][k]] = results[k]["out_t"].T.astype(np.float32)
    return out[: meta["N"]]


_LAST = None
_RUNNER_CACHE: dict = {}


class _PjrtRunner:
    """Builds the shard_map-jitted NEFF executor once; supports repeated
    dispatches with device-resident inputs for timing."""

    def __init__(self, nc):
        import jax
        import jax.numpy as jnp
        import concourse.mybir as mybir
        from concourse import bass2jax
        from jax.sharding import Mesh, NamedSharding, PartitionSpec
        from jax.experimental.shard_map import shard_map

        bass2jax.install_neuronx_cc_hook()
        self.jax = jax
        self.jnp = jnp
        in_names: list[str] = []
        out_names: list[str] = []
        out_avals = []
        out_shapes = []
        partition_name = (
            nc.partition_id_tensor.name if nc.partition_id_tensor else None
        )
        for alloc in nc.m.functions[0].allocations:
            if not isinstance(alloc, mybir.MemoryLocationSet):
                continue
            name = alloc.memorylocations[0].name
            if alloc.kind == "ExternalInput":
                if name != partition_name:
                    in_names.append(name)
            elif alloc.kind == "ExternalOutput":
                shape = tuple(alloc.tensor_shape)
                dtype = mybir.dt.np(alloc.dtype)
                out_names.append(name)
                out_avals.append(jax.core.ShapedArray(shape, dtype))
                out_shapes.append((shape, dtype))
        self.in_names = in_names
        self.out_names = out_names
        self.out_shapes = out_shapes
        n_params = len(in_names)
        n_outs = len(out_names)
        all_names = in_names + out_names
        if partition_name is not None:
            all_names = all_names + [partition_name]

        def _body(*args):
            operands = list(args)
            if partition_name is not None:
                operands.append(bass2jax.partition_id_tensor())
            outs = bass2jax._bass_exec_p.bind(
                *operands,
                out_avals=tuple(out_avals),
                in_names=tuple(all_names),
                out_names=tuple(out_names),
                lowering_input_output_aliases=(),
                sim_require_finite=True,
                sim_require_nnan=True,
                nc=nc,
            )
            return tuple(outs)

        devices = jax.devices()[:NCORES]
        self.mesh = Mesh(np.asarray(devices), ("core",))
        spec = PartitionSpec("core")
        self.sharding = NamedSharding(self.mesh, spec)
        self.sharded = jax.jit(
            shard_map(
                _body,
                mesh=self.mesh,
                in_specs=(spec,) * (n_params + n_outs),
                out_specs=(spec,) * n_outs,
                check_rep=False,
            ),
            donate_argnums=tuple(range(n_params, n_params + n_outs)),
            keep_unused=True,
        )

        def _mk_zeros():
            return tuple(
                jnp.zeros((NCORES * s[0], *s[1:]), d) for s, d in out_shapes
            )

        self.zeros_fn = jax.jit(
            _mk_zeros, out_shardings=(self.sharding,) * n_outs
        )

    def _stage_inputs(self, in_maps):
        concat = [
            np.concatenate(
                [np.asarray(in_maps[c][n]) for c in range(NCORES)], axis=0
            )
            for n in self.in_names
        ]
        return [self.jax.device_put(a, self.sharding) for a in concat]

    def _dispatch(self, staged):
        zeros = self.zeros_fn()
        outs = self.sharded(*staged, *zeros)
        self.jax.block_until_ready(outs)
        return outs

    def run(self, in_maps):
        staged = self._stage_inputs(in_maps)
        outs = self._dispatch(staged)
        res = []
        for c in range(NCORES):
            m = {}
            for i, n in enumerate(self.out_names):
                s, d = self.out_shapes[i]
                m[n] = np.asarray(outs[i]).reshape(NCORES, *s)[c]
            res.append(m)
        return res

    def timed(self, in_maps, iters=10):
        import time

        staged = self._stage_inputs(in_maps)
        self._dispatch(staged)  # warm
        walls = []
        for _ in range(iters):
            zeros = self.zeros_fn()
            self.jax.block_until_ready(zeros)
            t0 = time.perf_counter()
            outs = self.sharded(*staged, *zeros)
            self.jax.block_until_ready(outs)
            walls.append(time.perf_counter() - t0)
        tiny = self.jax.device_put(
            np.zeros((NCORES, 8), np.float32), self.sharding
        )
        base_fn = self.jax.jit(lambda a: a + 1.0)
        self.jax.block_until_ready(base_fn(tiny))
        bases = []
        for _ in range(iters):
            t0 = time.perf_counter()
            self.jax.block_until_ready(base_fn(tiny))
            bases.append(time.perf_counter() - t0)
        print(
            f"kernel walls min/med: {min(walls)*1e3:.2f}/"
            f"{np.median(walls)*1e3:.2f} ms; "
            f"baseline min/med: {min(bases)*1e3:.2f}/"
            f"{np.median(bases)*1e3:.2f} ms"
        )
        return max(float(np.median(walls) - np.median(bases)), 0.0) * 1e9


def _get_runner(nc):
    r = _RUNNER_CACHE.get(id(nc))
    if r is None:
        r = _PjrtRunner(nc)
        _RUNNER_CACHE[id(nc)] = r
    return r


def time_kernel(inputs=None, iters=8):
    assert _LAST is not None, "call kernel() first"
    nc, in_maps = _LAST
    return _get_runner(nc).timed(in_maps, iters=iters)
